# revision 1
# baseline (speedup 1.0000x reference)
"""Trainium2 Bass kernel for nn_AttentionBlock (MLA-style attention + SwiGLU FFN).

Self-contained: takes FULL inputs, shards across 8 NeuronCores internally,
returns FULL output.

Sharding:
  Launch 1 (attention): tensor-parallel over heads (2 heads/core); each core
    computes its heads' partial attn_out @ W_O slice; host sums partials.
  Launch 2 (FFN): 2D sharding (4 token-quarters x 2 ffn-halves); host sums
    the two ffn-half partials per token quarter.
All matmuls run in bf16 with fp32 PSUM accumulation. Softmax/normalization
statistics are computed in fp32. Activations arrive pre-transposed (host does
the [m,D]->[D,m] transpose), so the device never transposes.
"""
import sys
sys.path.insert(0, '/opt/trn_rl_repo')

import math
import numpy as np
import ml_dtypes

from concourse import bass, bacc, mybir, tile
from concourse.bass_utils import run_bass_kernel_spmd

# ---- inlined wait pruner (kernel.py must be self-contained) ----
import bisect


def _is_dma(inst):
    return type(inst).__name__ in (
        "InstDMACopy", "InstDmaTranspose", "InstDmaTransposeAnt",
        "InstTensorCopyDma", "InstTensorReduceDma")


def prune_redundant_waits(nc, verbose=False):
    insts = []
    for f in nc.m.functions:
        for blk in f.blocks:
            insts.extend(blk.instructions)

    poisoned = set()
    running = {}
    producers = {}   # sem -> ([values], [idx])
    VC = [None] * len(insts)
    chain_vc = {}    # engine -> completion vc of last instruction
    chain_prev = [None] * len(insts)   # vc inherited from chain (pre-wait)

    def producer_at_least(sem, v):
        if sem in poisoned or sem not in producers:
            return None
        vals, idxs = producers[sem]
        i = bisect.bisect_left(vals, v)
        if i == len(vals):
            return None
        return vals[i], idxs[i]

    def merge(dst, src):
        for s, v in src.items():
            if dst.get(s, -1) < v:
                dst[s] = v

    for idx, inst in enumerate(insts):
        si = inst.sync_info
        is_dma = _is_dma(inst)
        ekey = getattr(inst, "engine", None)
        if is_dma:
            vc = {}
        else:
            vc = dict(chain_vc.get(ekey, {}))
        chain_prev[idx] = dict(vc)
        if si is not None:
            for w in si.on_wait:
                if w.wait_mode != "sem-ge-imm" or w.id in poisoned:
                    continue
                p = producer_at_least(w.id, w.wait_value)
                if p is not None:
                    merge(vc, VC[p[1]])
                    if vc.get(w.id, -1) < p[0]:
                        vc[w.id] = p[0]
                else:
                    if vc.get(w.id, -1) < w.wait_value:
                        vc[w.id] = w.wait_value
            for u in si.on_update:
                if u.update_mode in ("sem-inc", "sem-add-imm"):
                    nv = running.get(u.id, 0) + u.update_value
                    running[u.id] = nv
                    producers.setdefault(u.id, ([], []))
                    producers[u.id][0].append(nv)
                    producers[u.id][1].append(idx)
                    if vc.get(u.id, -1) < nv:
                        vc[u.id] = nv
                else:
                    poisoned.add(u.id)
        VC[idx] = vc
        if not is_dma:
            chain_vc[ekey] = vc

    # pass 2: prune
    n_pruned = 0
    for idx, inst in enumerate(insts):
        si = inst.sync_info
        if si is None or len(si.on_wait) < 2:
            continue
        waits = list(si.on_wait)
        kept = list(waits)
        changed = True
        while changed and len(kept) > 1:
            changed = False
            for w in kept:
                if w.wait_mode != "sem-ge-imm" or w.id in poisoned:
                    continue
                cover = dict(chain_prev[idx])
                ok_others = True
                for o in kept:
                    if o is w:
                        continue
                    if o.wait_mode != "sem-ge-imm" or o.id in poisoned:
                        continue
                    p = producer_at_least(o.id, o.wait_value)
                    if p is not None:
                        merge(cover, VC[p[1]])
                if cover.get(w.id, -1) >= w.wait_value:
                    kept.remove(w)
                    n_pruned += 1
                    changed = True
                    break
        if len(kept) != len(waits):
            import concourse.mybir as mybir
            inst.sync_info = mybir.SyncInfo(on_wait=kept, on_update=list(si.on_update))
    if verbose:
        hist = {}
        for inst in insts:
            si = inst.sync_info
            n = len(si.on_wait) if si else 0
            k = (type(inst).__name__, n)
            hist[k] = hist.get(k, 0) + 1
        print(f"wait_prune: removed {n_pruned} waits; post histogram:",
              dict(sorted(hist.items())))
    return n_pruned

# ---- end wait pruner ----


BF = mybir.dt.bfloat16
F16 = mybir.dt.float16
F32 = mybir.dt.float32
AF = mybir.ActivationFunctionType

D = 2048
N_H = 16
D_H = 128
D_R = 64
FFN = 8192
THETA = 1000000.0
EPS = 1e-6
SCALE = 1.0 / math.sqrt(D_H + D_R)
NCORES = 8
P = 128
MB = 512


# --------------------------------------------------------------------------
# Launch 1: attention block, tensor-parallel over heads
# --------------------------------------------------------------------------
def build_attn(B, M, N, Dm, HPC, DH=D_H, DR=D_R):
    DC = Dm // P
    NT = N // P
    MT = M // P
    NBN = N // MB
    NBM = M // MB
    RD = HPC * DR
    HD = HPC * DH
    ln_scale_bias = float(math.log(SCALE))

    nc = bacc.Bacc()
    qT = nc.dram_tensor("qT", [B, Dm, M], BF, kind="ExternalInput")
    kvT = nc.dram_tensor("kvT", [B, Dm, N], BF, kind="ExternalInput")
    wq = nc.dram_tensor("wq", [Dm, HD], BF, kind="ExternalInput")
    wqr = nc.dram_tensor("wqr", [Dm, RD], BF, kind="ExternalInput")
    wk = nc.dram_tensor("wk", [Dm, HD], BF, kind="ExternalInput")
    wkr = nc.dram_tensor("wkr", [Dm, RD], BF, kind="ExternalInput")
    wv = nc.dram_tensor("wv", [Dm, HD], BF, kind="ExternalInput")
    wo = nc.dram_tensor("wo", [HD, Dm], BF, kind="ExternalInput")
    cos2T = nc.dram_tensor("cos2T", [RD, M], F16, kind="ExternalInput")
    sin2T = nc.dram_tensor("sin2T", [RD, M], F16, kind="ExternalInput")
    rot2T = nc.dram_tensor("rot2T", [RD, RD], BF, kind="ExternalInput")
    po = nc.dram_tensor("po", [B, M, Dm], F32, kind="ExternalOutput")

    with tile.TileContext(nc) as tc:
      with tc.tile_pool(name="const", bufs=1) as cp, \
           tc.tile_pool(name="dram", bufs=1, space="DRAM") as dramp:
        ones_bf = cp.tile([P, 1], BF, tag="ones")
        nc.vector.memset(ones_bf[:], 1.0)
        cosT_sb = cp.tile([RD, M], F16, tag="cos")
        sinT_sb = cp.tile([RD, M], F16, tag="sin")
        rot_sb = cp.tile([RD, RD], BF, tag="rot")
        eps_t = cp.tile([P, 1], F32, tag="eps")
        nc.vector.memset(eps_t[:], EPS)
        lnsc_t = cp.tile([P, 1], F32, tag="lnsc")
        nc.vector.memset(lnsc_t[:], ln_scale_bias)
        nc.sync.dma_start(out=cosT_sb[:], in_=cos2T[:])
        nc.sync.dma_start(out=sinT_sb[:], in_=sin2T[:])
        nc.sync.dma_start(out=rot_sb[:], in_=rot2T[:])

        for b in range(B):
          with tc.tile_pool(name=f"kq{b}", bufs=1) as kq:
            kt = [kq.tile([P, N], BF, tag=f"kt{h}", name=f"kt{h}") for h in range(HPC)]
            krt = kq.tile([RD, N], BF, tag="krt")
            vt = [kq.tile([P, HD], BF, tag=f"vt{i}", name=f"vt{i}") for i in range(NT)]
            qt = [kq.tile([P, M], BF, tag=f"qt{h}", name=f"qt{h}") for h in range(HPC)]
            qrt = kq.tile([RD, M], BF, tag="qrt")
            nkv_col = kq.tile([P, NT], F32, tag="nkvc")
            nkvV_col = kq.tile([P, NT], F32, tag="nkvvc")
            nq_bc = kq.tile([P, M], F32, tag="nqbc")

            # ================= KV side =================
            with tc.tile_pool(name=f"kvw{b}", bufs=1) as wp, \
                 tc.tile_pool(name=f"kvs{b}", bufs=2) as sp:
                wkt = [wp.tile([P, HD], BF, tag=f"wk{i}", name=f"wk{i}") for i in range(DC)]
                wkrt = [wp.tile([P, RD], BF, tag=f"wkr{i}", name=f"wkr{i}") for i in range(DC)]
                wvt = [wp.tile([P, HD], BF, tag=f"wv{i}", name=f"wv{i}") for i in range(DC)]
                kv_sb = [wp.tile([P, N], BF, tag=f"akv{i}", name=f"akv{i}") for i in range(DC)]
                for dc in range(DC):
                    nc.sync.dma_start(out=wkt[dc][:], in_=wk[dc * P:(dc + 1) * P, :])
                    nc.sync.dma_start(out=wkrt[dc][:], in_=wkr[dc * P:(dc + 1) * P, :])
                    nc.sync.dma_start(out=wvt[dc][:], in_=wv[dc * P:(dc + 1) * P, :])
                    nc.sync.dma_start(out=kv_sb[dc][:], in_=kvT[b, dc * P:(dc + 1) * P, :])

                # --- rms stats: sum_d(x^2) via Square + ones-matmul ---
                with tc.tile_pool(name=f"kvn{b}", bufs=1, space="PSUM") as pn:
                    sumsq = [pn.tile([1, MB], F32, tag=f"ss{nb}", name=f"ss{nb}") for nb in range(NBN)]
                    for dc in range(DC):
                        sq = sp.tile([P, N], BF, tag="sq")
                        nc.scalar.activation(sq[:], kv_sb[dc][:], AF.Square)
                        for nb in range(NBN):
                            nc.tensor.matmul(sumsq[nb][:], ones_bf[:],
                                             sq[:, nb * MB:(nb + 1) * MB],
                                             start=(dc == 0), stop=(dc == DC - 1))
                    ln_row = sp.tile([1, N], F32, tag="lnrow")
                    for nb in range(NBN):
                        nc.scalar.activation(ln_row[0:1, nb * MB:(nb + 1) * MB],
                                             sumsq[nb][:], AF.Ln,
                                             scale=1.0 / Dm, bias=eps_t[0:1, :])
                drow = dramp.tile([1, N], F32, tag=f"dkv{b}")
                nc.sync.dma_start(out=drow[:], in_=ln_row[:])
                lncol = sp.tile([P, NT], F32, tag="lncol")
                nc.sync.dma_start(out=lncol[:],
                                  in_=drow[:].rearrange("a (t p) -> (a p) t", p=P))
                nc.scalar.activation(nkv_col[:], lncol[:], AF.Exp,
                                     scale=-0.5, bias=lnsc_t[:])
                nc.scalar.activation(nkvV_col[:], lncol[:], AF.Exp, scale=-0.5)

                pp_cm = tc.tile_pool(name=f"kvp{b}", bufs=2, space="PSUM")
                pp = pp_cm.__enter__()
                # --- K_C^T projection ---
                for h in range(HPC):
                    for nb in range(NBN):
                        ps = pp.tile([P, MB], F32, tag="proj")
                        for dc in range(DC):
                            nc.tensor.matmul(ps[:], wkt[dc][:, h * DH:(h + 1) * DH],
                                             kv_sb[dc][:, nb * MB:(nb + 1) * MB],
                                             start=(dc == 0), stop=(dc == DC - 1))
                        nc.scalar.copy(kt[h][:, nb * MB:(nb + 1) * MB], ps[:])
                # --- K_R^T projection (heads stacked on partitions) ---
                krt_raw = sp.tile([RD, N], BF, tag="krraw")
                for nb in range(NBN):
                    ps = pp.tile([RD, MB], F32, tag="projr")
                    for dc in range(DC):
                        nc.tensor.matmul(ps[:], wkrt[dc][:],
                                         kv_sb[dc][:, nb * MB:(nb + 1) * MB],
                                         start=(dc == 0), stop=(dc == DC - 1))
                    nc.scalar.copy(krt_raw[:, nb * MB:(nb + 1) * MB], ps[:])
                # --- V projection (activations stationary, nkv-scaled evac) ---
                for nt in range(NT):
                    ps = pp.tile([P, HD], F32, tag="projv")
                    for dc in range(DC):
                        nc.tensor.matmul(ps[:], kv_sb[dc][:, nt * P:(nt + 1) * P],
                                         wvt[dc][:],
                                         start=(dc == 0), stop=(dc == DC - 1))
                    nc.vector.tensor_scalar_mul(vt[nt][:], ps[:],
                                                nkvV_col[:, nt:nt + 1])
                # --- rope K ---
                for nb in range(NBN):
                    nbs = slice(nb * MB, (nb + 1) * MB)
                    rps = pp.tile([RD, MB], F32, tag="rot")
                    nc.tensor.matmul(rps[:], rot_sb[:], krt_raw[:, nbs],
                                     start=True, stop=True)
                    c_t = sp.tile([RD, MB], BF, tag="ropec")
                    nc.vector.tensor_mul(c_t[:], krt_raw[:, nbs], cosT_sb[:, nbs])
                    s_t = sp.tile([RD, MB], BF, tag="ropes")
                    nc.vector.tensor_mul(s_t[:], rps[:], sinT_sb[:, nbs])
                    nc.vector.tensor_add(krt[:, nbs], c_t[:], s_t[:])
                pp_cm.__exit__(None, None, None)

            # ================= Q side =================
            with tc.tile_pool(name=f"qw{b}", bufs=1) as wp, \
                 tc.tile_pool(name=f"qs{b}", bufs=2) as sp:
                wqt = [wp.tile([P, HD], BF, tag=f"wq{i}", name=f"wq{i}") for i in range(DC)]
                wqrt = [wp.tile([P, RD], BF, tag=f"wqr{i}", name=f"wqr{i}") for i in range(DC)]
                q_sb = [wp.tile([P, M], BF, tag=f"aq{i}", name=f"aq{i}") for i in range(DC)]
                for dc in range(DC):
                    nc.sync.dma_start(out=wqt[dc][:], in_=wq[dc * P:(dc + 1) * P, :])
                    nc.sync.dma_start(out=wqrt[dc][:], in_=wqr[dc * P:(dc + 1) * P, :])
                    nc.sync.dma_start(out=q_sb[dc][:], in_=qT[b, dc * P:(dc + 1) * P, :])

                with tc.tile_pool(name=f"qn{b}", bufs=1, space="PSUM") as pn:
                    sumsq = [pn.tile([1, MB], F32, tag=f"ss{nb}", name=f"ss{nb}") for nb in range(NBM)]
                    for dc in range(DC):
                        sq = sp.tile([P, M], BF, tag="sq")
                        nc.scalar.activation(sq[:], q_sb[dc][:], AF.Square)
                        for nb in range(NBM):
                            nc.tensor.matmul(sumsq[nb][:], ones_bf[:],
                                             sq[:, nb * MB:(nb + 1) * MB],
                                             start=(dc == 0), stop=(dc == DC - 1))
                    ln_row = sp.tile([1, M], F32, tag="lnrow")
                    for nb in range(NBM):
                        nc.scalar.activation(ln_row[0:1, nb * MB:(nb + 1) * MB],
                                             sumsq[nb][:], AF.Ln,
                                             scale=1.0 / Dm, bias=eps_t[0:1, :])
                    nq_row = sp.tile([1, M], F32, tag="nqrow")
                    nc.scalar.activation(nq_row[:], ln_row[:], AF.Exp, scale=-0.5)
                drow = dramp.tile([1, M], F32, tag=f"dq{b}")
                nc.sync.dma_start(out=drow[:], in_=nq_row[:])
                nc.sync.dma_start(out=nq_bc[:], in_=drow[:].to_broadcast((P, M)))
                pp_cm = tc.tile_pool(name=f"qp{b}", bufs=2, space="PSUM")
                pp = pp_cm.__enter__()

                for h in range(HPC):
                    for nb in range(NBM):
                        nbs = slice(nb * MB, (nb + 1) * MB)
                        ps = pp.tile([P, MB], F32, tag="proj")
                        for dc in range(DC):
                            nc.tensor.matmul(ps[:], wqt[dc][:, h * DH:(h + 1) * DH],
                                             q_sb[dc][:, nbs],
                                             start=(dc == 0), stop=(dc == DC - 1))
                        nc.vector.tensor_mul(qt[h][:, nbs], ps[:], nq_bc[:, nbs])
                qrt_raw = sp.tile([RD, M], BF, tag="qrraw")
                for nb in range(NBM):
                    nbs = slice(nb * MB, (nb + 1) * MB)
                    ps = pp.tile([RD, MB], F32, tag="projr")
                    for dc in range(DC):
                        nc.tensor.matmul(ps[:], wqrt[dc][:], q_sb[dc][:, nbs],
                                         start=(dc == 0), stop=(dc == DC - 1))
                    nc.vector.tensor_mul(qrt_raw[:, nbs], ps[:], nq_bc[:RD, nbs])
                for nb in range(NBM):
                    nbs = slice(nb * MB, (nb + 1) * MB)
                    rps = pp.tile([RD, MB], F32, tag="rot")
                    nc.tensor.matmul(rps[:], rot_sb[:], qrt_raw[:, nbs],
                                     start=True, stop=True)
                    c_t = sp.tile([RD, MB], BF, tag="ropec")
                    nc.vector.tensor_mul(c_t[:], qrt_raw[:, nbs], cosT_sb[:, nbs])
                    s_t = sp.tile([RD, MB], BF, tag="ropes")
                    nc.vector.tensor_mul(s_t[:], rps[:], sinT_sb[:, nbs])
                    nc.vector.tensor_add(qrt[:, nbs], c_t[:], s_t[:])
                pp_cm.__exit__(None, None, None)

            # ================= attention + W_O =================
            with tc.tile_pool(name=f"at{b}", bufs=1) as ap, \
                 tc.tile_pool(name=f"ap{b}", bufs=2, space="PSUM") as pp, \
                 tc.tile_pool(name=f"ae{b}", bufs=2 * NT + 2) as ep, \
                 tc.tile_pool(name=f"as{b}", bufs=2) as sp:
                ut = [ap.tile([P, M], BF, tag=f"ut{h}", name=f"ut{h}") for h in range(HPC)]
                wo_sb = [ap.tile([P, Dm], BF, tag=f"wo{h}", name=f"wo{h}") for h in range(HPC)]
                for h in range(HPC):
                    nc.sync.dma_start(out=wo_sb[h][:], in_=wo[h * DH:(h + 1) * DH, :])
                drs = dramp.tile([1, M], F32, tag=f"drs{b}")

                for h in range(HPC):
                    for mb in range(NBM):
                        mbs = slice(mb * MB, (mb + 1) * MB)
                        u_ps = pp.tile([P, MB], F32, tag="u")
                        sum_ps = pp.tile([1, MB], F32, tag="sums")
                        for nt in range(NT):
                            s_ps = pp.tile([P, MB], F32, tag="s")
                            nc.tensor.matmul(s_ps[:], kt[h][:, nt * P:(nt + 1) * P],
                                             qt[h][:, mbs], start=True, stop=False)
                            nc.tensor.matmul(
                                s_ps[:],
                                krt[h * DR:(h + 1) * DR, nt * P:(nt + 1) * P],
                                qrt[h * DR:(h + 1) * DR, mbs],
                                start=False, stop=True)
                            et = ep.tile([P, MB], BF, tag="et")
                            nc.scalar.activation(et[:], s_ps[:], AF.Exp,
                                                 scale=nkv_col[:, nt:nt + 1])
                            nc.tensor.matmul(u_ps[:], vt[nt][:, h * DH:(h + 1) * DH],
                                             et[:], start=(nt == 0),
                                             stop=(nt == NT - 1))
                            nc.tensor.matmul(sum_ps[:], ones_bf[:], et[:],
                                             start=(nt == 0), stop=(nt == NT - 1))
                        rs_row = sp.tile([1, MB], F32, tag="rs")
                        nc.vector.reciprocal(rs_row[:], sum_ps[:])
                        nc.sync.dma_start(out=drs[0:1, mbs], in_=rs_row[:])
                        rsb = sp.tile([P, MB], F32, tag="rsb")
                        nc.sync.dma_start(out=rsb[:],
                                          in_=drs[0:1, mbs].to_broadcast((P, MB)))
                        nc.vector.tensor_mul(ut[h][:, mbs], u_ps[:], rsb[:])

                for mt in range(MT):
                    po_sb = sp.tile([P, Dm], F32, tag="po")
                    for ocb in range(Dm // MB):
                        w_ps = pp.tile([P, MB], F32, tag="wops")
                        for h in range(HPC):
                            nc.tensor.matmul(w_ps[:], ut[h][:, mt * P:(mt + 1) * P],
                                             wo_sb[h][:, ocb * MB:(ocb + 1) * MB],
                                             start=(h == 0), stop=(h == HPC - 1))
                        nc.scalar.copy(po_sb[:, ocb * MB:(ocb + 1) * MB], w_ps[:])
                    nc.sync.dma_start(out=po[b, mt * P:(mt + 1) * P, :], in_=po_sb[:])
    prune_redundant_waits(nc, verbose=True)
    nc.compile()
    return nc


# --------------------------------------------------------------------------
# Launch 2: FFN, token-quarter x ffn-half sharding
# --------------------------------------------------------------------------
def build_ffn(TOK, Dm, FH, act_fn=None):
    DC = Dm // P
    FC = FH // P
    NBM = TOK // MB
    MTT = TOK // P

    nc = bacc.Bacc()
    xnT = nc.dram_tensor("xnT", [Dm, TOK], BF, kind="ExternalInput")
    wg = nc.dram_tensor("wg", [Dm, FH], BF, kind="ExternalInput")
    wu = nc.dram_tensor("wu", [Dm, FH], BF, kind="ExternalInput")
    wd = nc.dram_tensor("wd", [FH, Dm], BF, kind="ExternalInput")
    fo = nc.dram_tensor("fo", [TOK, Dm], F32, kind="ExternalOutput")

    with tile.TileContext(nc) as tc:
      with tc.tile_pool(name="xp", bufs=1) as xp, \
           tc.tile_pool(name="hp", bufs=1) as hp:
        xn_sb = [xp.tile([P, TOK], BF, tag=f"xn{i}", name=f"xn{i}") for i in range(DC)]
        for dc in range(DC):
            nc.sync.dma_start(out=xn_sb[dc][:], in_=xnT[dc * P:(dc + 1) * P, :])
        ht = [hp.tile([P, TOK], BF, tag=f"h{i}", name=f"h{i}") for i in range(FC)]

        with tc.tile_pool(name="gw", bufs=4) as gw, \
             tc.tile_pool(name="gp", bufs=2, space="PSUM") as gps, \
             tc.tile_pool(name="gs", bufs=3) as gsp:
            for fc in range(FC):
                g_ps = gps.tile([P, TOK], F32, tag="g")
                u_ps = gps.tile([P, TOK], F32, tag="u")
                for dc in range(DC):
                    wgt = gw.tile([P, P], BF, tag="wg")
                    wut = gw.tile([P, P], BF, tag="wu")
                    nc.sync.dma_start(
                        out=wgt[:], in_=wg[dc * P:(dc + 1) * P, fc * P:(fc + 1) * P])
                    nc.sync.dma_start(
                        out=wut[:], in_=wu[dc * P:(dc + 1) * P, fc * P:(fc + 1) * P])
                    for nb in range(NBM):
                        mbs = slice(nb * MB, (nb + 1) * MB)
                        nc.tensor.matmul(g_ps[:, mbs], wgt[:], xn_sb[dc][:, mbs],
                                         start=(dc == 0), stop=(dc == DC - 1))
                        nc.tensor.matmul(u_ps[:, mbs], wut[:], xn_sb[dc][:, mbs],
                                         start=(dc == 0), stop=(dc == DC - 1))
                hs = gsp.tile([P, TOK], BF, tag="hs")
                nc.scalar.activation(hs[:], g_ps[:],
                                 AF.Silu if act_fn is None else act_fn)
                nc.vector.tensor_mul(ht[fc][:], hs[:], u_ps[:])

        with tc.tile_pool(name="dw", bufs=2) as dw, \
             tc.tile_pool(name="dp", bufs=4, space="PSUM") as dps, \
             tc.tile_pool(name="ds", bufs=3) as dsp:
            for ocb in range(Dm // MB):
                ocs = slice(ocb * MB, (ocb + 1) * MB)
                wdt = [dw.tile([P, MB], BF, tag=f"wd{fc}", name=f"wd{fc}") for fc in range(FC)]
                for fc in range(FC):
                    nc.sync.dma_start(out=wdt[fc][:], in_=wd[fc * P:(fc + 1) * P, ocs])
                for mt in range(MTT):
                    d_ps = dps.tile([P, MB], F32, tag="d")
                    for fc in range(FC):
                        nc.tensor.matmul(d_ps[:], ht[fc][:, mt * P:(mt + 1) * P],
                                         wdt[fc][:],
                                         start=(fc == 0), stop=(fc == FC - 1))
                    o_sb = dsp.tile([P, MB], F32, tag="o")
                    nc.scalar.copy(o_sb[:], d_ps[:])
                    nc.sync.dma_start(out=fo[mt * P:(mt + 1) * P, ocs], in_=o_sb[:])
    prune_redundant_waits(nc, verbose=True)
    nc.compile()
    return nc


# --------------------------------------------------------------------------
# Host orchestration
# --------------------------------------------------------------------------
_prog_cache = {}


def _get(key, builder, *args):
    if key not in _prog_cache:
        _prog_cache[key] = builder(*args)
    return _prog_cache[key]


def _bf(x):
    return np.ascontiguousarray(np.asarray(x, dtype=np.float32)).astype(
        ml_dtypes.bfloat16)


def _rope_tables(S, dim):
    freqs = 1.0 / (THETA ** (np.arange(0, dim, 2, dtype=np.float32) / dim))
    f = np.arange(S, dtype=np.float32)[:, None] * freqs[None, :]
    cos = np.repeat(np.cos(f), 2, axis=-1).astype(np.float32)
    sin = np.repeat(np.sin(f), 2, axis=-1).astype(np.float32)
    return cos, sin


def _rot_lhsT(dim):
    rt = np.zeros((dim, dim), np.float32)
    for i in range(dim // 2):
        rt[2 * i + 1, 2 * i] = -1.0
        rt[2 * i, 2 * i + 1] = 1.0
    return rt




def _timed_run(nc, in_maps, reps=10):
    """Execute on all cores with device-resident inputs; time warm reps.

    Returns (results_list, best_exec_seconds). Mimics
    bass2jax.run_bass_via_pjrt's multi-core path but keeps inputs on
    device so the timed region is pure NEFF execution + dispatch.
    """
    import time as _time
    import jax
    from jax.sharding import Mesh, PartitionSpec, NamedSharding
    from jax.experimental.shard_map import shard_map
    from concourse import bass2jax as b2j
    from concourse import mybir as _mb

    b2j.install_neuronx_cc_hook()
    n_cores = len(in_maps)
    in_names, out_names, out_avals, zero_outs = [], [], [], []
    for alloc in nc.m.functions[0].allocations:
        if not isinstance(alloc, _mb.MemoryLocationSet):
            continue
        name = alloc.memorylocations[0].name
        pid_name = nc.partition_id_tensor.name if nc.partition_id_tensor else None
        if alloc.kind == "ExternalInput":
            if name != pid_name:
                in_names.append(name)
        elif alloc.kind == "ExternalOutput":
            out_names.append(name)
            shape = tuple(alloc.tensor_shape)
            dtype = _mb.dt.np(alloc.dtype)
            out_avals.append(jax.core.ShapedArray(shape, dtype))
            zero_outs.append(np.zeros(shape, dtype))
    n_params = len(in_names)
    n_outs = len(out_avals)
    all_names = list(in_names) + list(out_names)
    if nc.partition_id_tensor is not None:
        all_names.append(nc.partition_id_tensor.name)

    def _body(*args):
        operands = list(args)
        if nc.partition_id_tensor is not None:
            operands.append(b2j.partition_id_tensor())
        outs = b2j._bass_exec_p.bind(
            *operands, out_avals=tuple(out_avals), in_names=tuple(all_names),
            out_names=tuple(out_names), lowering_input_output_aliases=(),
            sim_require_finite=True, sim_require_nnan=True, nc=nc)
        return tuple(outs)

    devices = jax.devices()[:n_cores]
    mesh = Mesh(np.asarray(devices), ("core",))
    donate = tuple(range(n_params, n_params + n_outs))
    sharded = jax.jit(
        shard_map(_body, mesh=mesh,
                  in_specs=(PartitionSpec("core"),) * (n_params + n_outs),
                  out_specs=(PartitionSpec("core"),) * n_outs,
                  check_rep=False),
        donate_argnums=donate, keep_unused=True)
    sh = NamedSharding(mesh, PartitionSpec("core"))
    dev_in = [jax.device_put(
        np.concatenate([np.asarray(in_maps[c][n]) for c in range(n_cores)], axis=0), sh)
        for n in in_names]
    # warmup + correctness outputs
    dz = [jax.device_put(
        np.zeros((n_cores * z.shape[0], *z.shape[1:]), z.dtype), sh)
        for z in zero_outs]
    jax.block_until_ready(dz)
    outs = sharded(*dev_in, *dz)
    jax.block_until_ready(outs)
    # pipelined timing: amortize per-call dispatch overhead over reps
    zsets = [[jax.device_put(
        np.zeros((n_cores * z.shape[0], *z.shape[1:]), z.dtype), sh)
        for z in zero_outs] for _ in range(reps)]
    jax.block_until_ready(zsets)
    t0 = _time.perf_counter()
    last = None
    for k in range(reps):
        last = sharded(*dev_in, *zsets[k])
    jax.block_until_ready(last)
    total = _time.perf_counter() - t0
    best = total / reps
    results = [
        {name: np.asarray(outs[i]).reshape(n_cores, *out_avals[i].shape)[c]
         for i, name in enumerate(out_names)}
        for c in range(n_cores)]
    return results, best


_last_exec_ns = []


class _Res:
    def __init__(self, results):
        self.results = results


def _run(nc, in_maps, trace=False):
    if trace:
        results, secs = _timed_run(nc, in_maps)
        _last_exec_ns.append(int(secs * 1e9))
        return _Res(results)
    res = run_bass_kernel_spmd(nc, in_maps, list(range(len(in_maps))))
    _last_exec_ns.append(res.exec_time_ns)
    return res


def kernel(query, key_value, g_q, g_kv, g_ffn, w_qc, w_kc, w_qr, w_kr, w_v,
           w_o, w_gate, w_up, w_down, _trace=False):
    query = np.asarray(query, np.float32)
    key_value = np.asarray(key_value, np.float32)
    Bq, Mq, _ = query.shape
    Nq = key_value.shape[1]
    HPC = N_H // NCORES

    g_q = np.asarray(g_q, np.float32)[:, None]
    g_kv = np.asarray(g_kv, np.float32)[:, None]
    g_ffn = np.asarray(g_ffn, np.float32)[:, None]
    wqc = np.asarray(w_qc, np.float32) * g_q
    wqr_f = np.asarray(w_qr, np.float32) * g_q
    wkc = np.asarray(w_kc, np.float32) * g_kv
    wkr_f = np.asarray(w_kr, np.float32) * g_kv
    wv_f = np.asarray(w_v, np.float32) * g_kv
    wo_f = np.asarray(w_o, np.float32)
    wgate = np.asarray(w_gate, np.float32) * g_ffn
    wup = np.asarray(w_up, np.float32) * g_ffn
    wdown = np.asarray(w_down, np.float32)

    qT = _bf(query.transpose(0, 2, 1))
    kvT = _bf(key_value.transpose(0, 2, 1))
    cos, sin = _rope_tables(max(Mq, Nq), D_R)
    cos2T = np.ascontiguousarray(np.vstack([cos[:Mq].T] * HPC)).astype(np.float16)
    sin2T = np.ascontiguousarray(np.vstack([sin[:Mq].T] * HPC)).astype(np.float16)
    rot2T = _bf(np.kron(np.eye(HPC, dtype=np.float32), _rot_lhsT(D_R)))

    del _last_exec_ns[:]
    nc1 = _get(("attn", Bq, Mq, Nq, D, HPC), build_attn, Bq, Mq, Nq, D, HPC)
    in_maps = []
    for c in range(NCORES):
        hs = slice(c * HPC * D_H, (c + 1) * HPC * D_H)
        rs = slice(c * HPC * D_R, (c + 1) * HPC * D_R)
        in_maps.append({
            "qT": qT, "kvT": kvT,
            "wq": _bf(wqc[:, hs]), "wqr": _bf(wqr_f[:, rs]),
            "wk": _bf(wkc[:, hs]), "wkr": _bf(wkr_f[:, rs]),
            "wv": _bf(wv_f[:, hs]), "wo": _bf(wo_f[hs, :]),
            "cos2T": cos2T, "sin2T": sin2T, "rot2T": rot2T,
        })
    res1 = _run(nc1, in_maps, trace=_trace)
    attn = np.zeros((Bq, Mq, D), np.float32)
    for r in res1.results:
        attn += r["po"]

    x = query + attn
    xf = x.reshape(Bq * Mq, D)
    n = 1.0 / np.sqrt((xf * xf).mean(axis=-1, keepdims=True) + EPS)
    xn = xf * n
    TQ = 4
    FHALF = FFN // 2
    tok = Bq * Mq // TQ
    xnT_sh = [_bf(xn[t * tok:(t + 1) * tok, :].T) for t in range(TQ)]
    wg_h = [_bf(wgate[:, :FHALF]), _bf(wgate[:, FHALF:])]
    wu_h = [_bf(wup[:, :FHALF]), _bf(wup[:, FHALF:])]
    wd_h = [_bf(wdown[:FHALF, :]), _bf(wdown[FHALF:, :])]

    nc2 = _get(("ffn", tok, D, FHALF), build_ffn, tok, D, FHALF)
    in_maps2 = []
    for c in range(NCORES):
        tq, fh = c % TQ, c // TQ
        in_maps2.append({"xnT": xnT_sh[tq], "wg": wg_h[fh], "wu": wu_h[fh],
                         "wd": wd_h[fh]})
    res2 = _run(nc2, in_maps2, trace=_trace)

    ffn_out = np.zeros((Bq * Mq, D), np.float32)
    for c in range(NCORES):
        tq = c % TQ
        ffn_out[tq * tok:(tq + 1) * tok, :] += res2.results[c]["fo"]

    y = x + ffn_out.reshape(Bq, Mq, D)
    return y



# revision 4
# speedup vs baseline: 2.0213x; 2.0213x over previous
"""Trainium2 Bass kernel for nn_AttentionBlock (MLA-style attention + SwiGLU FFN).

Self-contained: takes FULL inputs, shards across 8 NeuronCores internally,
returns FULL output.

Sharding (single fused launch, zero collectives):
  Each core owns 512 query tokens (batch b = c//4, quarter tq = c%4) and
  computes the WHOLE block for them: KV projections for its batch (all 16
  heads, duplicated across the 4 cores sharing a batch - cheaper than any
  cross-core reduction here), attention in head-pair groups, W_O + residual,
  RMS norm, token-local FFN, final residual. Output is the core's yT
  [D, 512] f32 slice; host transposes and concatenates.
All matmuls run in bf16 with fp32 PSUM accumulation, moving-operand free
size 512. Softmax/norm statistics are fp32 (rsqrt via the Ln+Exp table so
the whole kernel needs one activation-table switch, to Silu, at the FFN).
"""
import sys
sys.path.insert(0, '/opt/trn_rl_repo')

import math
import numpy as np
import ml_dtypes

from concourse import bass, bacc, mybir, tile
from concourse.bass_utils import run_bass_kernel_spmd

# ---- inlined wait pruner (kernel.py must be self-contained) ----
import bisect


def _is_dma(inst):
    return type(inst).__name__ in (
        "InstDMACopy", "InstDmaTranspose", "InstDmaTransposeAnt",
        "InstTensorCopyDma", "InstTensorReduceDma")


def prune_redundant_waits(nc, verbose=False):
    insts = []
    for f in nc.m.functions:
        for blk in f.blocks:
            insts.extend(blk.instructions)

    poisoned = set()
    running = {}
    producers = {}   # sem -> ([values], [idx])
    VC = [None] * len(insts)
    chain_vc = {}    # engine -> completion vc of last instruction
    chain_prev = [None] * len(insts)   # vc inherited from chain (pre-wait)

    def producer_at_least(sem, v):
        if sem in poisoned or sem not in producers:
            return None
        vals, idxs = producers[sem]
        i = bisect.bisect_left(vals, v)
        if i == len(vals):
            return None
        return vals[i], idxs[i]

    def merge(dst, src):
        for s, v in src.items():
            if dst.get(s, -1) < v:
                dst[s] = v

    for idx, inst in enumerate(insts):
        si = inst.sync_info
        is_dma = _is_dma(inst)
        ekey = getattr(inst, "engine", None)
        if is_dma:
            vc = {}
        else:
            vc = dict(chain_vc.get(ekey, {}))
        chain_prev[idx] = dict(vc)
        if si is not None:
            for w in si.on_wait:
                if w.wait_mode != "sem-ge-imm" or w.id in poisoned:
                    continue
                p = producer_at_least(w.id, w.wait_value)
                if p is not None:
                    merge(vc, VC[p[1]])
                    if vc.get(w.id, -1) < p[0]:
                        vc[w.id] = p[0]
                else:
                    if vc.get(w.id, -1) < w.wait_value:
                        vc[w.id] = w.wait_value
            for u in si.on_update:
                if u.update_mode in ("sem-inc", "sem-add-imm"):
                    nv = running.get(u.id, 0) + u.update_value
                    running[u.id] = nv
                    producers.setdefault(u.id, ([], []))
                    producers[u.id][0].append(nv)
                    producers[u.id][1].append(idx)
                    if vc.get(u.id, -1) < nv:
                        vc[u.id] = nv
                else:
                    poisoned.add(u.id)
        VC[idx] = vc
        if not is_dma:
            chain_vc[ekey] = vc

    # pass 2: prune
    n_pruned = 0
    for idx, inst in enumerate(insts):
        si = inst.sync_info
        if si is None or len(si.on_wait) < 2:
            continue
        waits = list(si.on_wait)
        kept = list(waits)
        changed = True
        while changed and len(kept) > 1:
            changed = False
            for w in kept:
                if w.wait_mode != "sem-ge-imm" or w.id in poisoned:
                    continue
                cover = dict(chain_prev[idx])
                ok_others = True
                for o in kept:
                    if o is w:
                        continue
                    if o.wait_mode != "sem-ge-imm" or o.id in poisoned:
                        continue
                    p = producer_at_least(o.id, o.wait_value)
                    if p is not None:
                        merge(cover, VC[p[1]])
                if cover.get(w.id, -1) >= w.wait_value:
                    kept.remove(w)
                    n_pruned += 1
                    changed = True
                    break
        if len(kept) != len(waits):
            import concourse.mybir as mybir
            inst.sync_info = mybir.SyncInfo(on_wait=kept, on_update=list(si.on_update))
    if verbose:
        print(f"wait_prune: removed {n_pruned} waits over {len(insts)} insts")
    return n_pruned

# ---- end wait pruner ----


BF = mybir.dt.bfloat16
F16 = mybir.dt.float16
F32 = mybir.dt.float32
AF = mybir.ActivationFunctionType

D = 2048
N_H = 16
D_H = 128
D_R = 64
FFN = 8192
THETA = 1000000.0
EPS = 1e-6
SCALE = 1.0 / math.sqrt(D_H + D_R)
NCORES = 8
P = 128
MB = 512
TOK = 512            # query tokens per core
G2 = 2               # heads per group (one rope pair-tile)


# --------------------------------------------------------------------------
# Fused single-launch kernel builder
# --------------------------------------------------------------------------
def build_fused(N=2048, Dm=D):
    NG = N_H // G2       # 8 head groups
    DC = Dm // P         # 16 contraction chunks
    NT = N // P          # 16 key tiles
    NBN = N // MB        # 4 key blocks of 512
    DTT = Dm // P        # 16 output d-tiles
    FC = FFN // P        # 64 ffn chunks
    GH = G2 * D_H        # 256 cols per group (K_C/Q_C/V)
    ln_scale_bias = float(math.log(SCALE))

    nc = bacc.Bacc()
    kvT = nc.dram_tensor("kvT", [Dm, N], BF, kind="ExternalInput")
    qT = nc.dram_tensor("qT", [Dm, TOK], BF, kind="ExternalInput")
    qres = nc.dram_tensor("qres", [Dm, TOK], F32, kind="ExternalInput")
    wkp = nc.dram_tensor("wkp", [NG, P, DC * GH], BF, kind="ExternalInput")
    wqp = nc.dram_tensor("wqp", [NG, P, DC * GH], BF, kind="ExternalInput")
    wvp = nc.dram_tensor("wvp", [NG, P, DC * GH], BF, kind="ExternalInput")
    wkrp = nc.dram_tensor("wkrp", [NG, P, DC * P], BF, kind="ExternalInput")
    wqrp = nc.dram_tensor("wqrp", [NG, P, DC * P], BF, kind="ExternalInput")
    wop = nc.dram_tensor("wop", [N_H, P, Dm], BF, kind="ExternalInput")
    wgp = nc.dram_tensor("wgp", [FC, P, DC * P], BF, kind="ExternalInput")
    wup = nc.dram_tensor("wup", [FC, P, DC * P], BF, kind="ExternalInput")
    wdp = nc.dram_tensor("wdp", [DTT, P, FC * P], BF, kind="ExternalInput")
    cosK = nc.dram_tensor("cosK", [P, N], F16, kind="ExternalInput")
    sinK = nc.dram_tensor("sinK", [P, N], F16, kind="ExternalInput")
    cosQ = nc.dram_tensor("cosQ", [P, TOK], F16, kind="ExternalInput")
    sinQ = nc.dram_tensor("sinQ", [P, TOK], F16, kind="ExternalInput")
    rot2 = nc.dram_tensor("rot2", [P, P], BF, kind="ExternalInput")
    yT = nc.dram_tensor("yT", [Dm, TOK], F32, kind="ExternalOutput")

    with tile.TileContext(nc) as tc:
      with tc.tile_pool(name="const", bufs=1) as cp, \
           tc.tile_pool(name="dram", bufs=1, space="DRAM") as dramp, \
           tc.tile_pool(name="outer", bufs=1) as op:
        ones_bf = cp.tile([P, 1], BF, tag="ones")
        nc.vector.memset(ones_bf[:], 1.0)
        eps_t = cp.tile([P, 1], F32, tag="eps")
        nc.vector.memset(eps_t[:], EPS)
        lnsc_t = cp.tile([P, 1], F32, tag="lnsc")
        nc.vector.memset(lnsc_t[:], ln_scale_bias)
        cosK_sb = cp.tile([P, N], F16, tag="cosK")
        sinK_sb = cp.tile([P, N], F16, tag="sinK")
        cosQ_sb = cp.tile([P, TOK], F16, tag="cosQ")
        sinQ_sb = cp.tile([P, TOK], F16, tag="sinQ")
        rot_sb = cp.tile([P, P], BF, tag="rot")
        nc.sync.dma_start(out=cosK_sb[:], in_=cosK[:])
        nc.sync.dma_start(out=sinK_sb[:], in_=sinK[:])
        nc.sync.dma_start(out=cosQ_sb[:], in_=cosQ[:])
        nc.sync.dma_start(out=sinQ_sb[:], in_=sinQ[:])
        nc.sync.dma_start(out=rot_sb[:], in_=rot2[:])

        ut = [op.tile([P, TOK], BF, tag=f"ut{h}", name=f"ut{h}")
              for h in range(N_H)]
        nkv_col = op.tile([P, NT], F32, tag="nkvc")
        nkvV_col = op.tile([P, NT], F32, tag="nkvvc")
        nq_bc = op.tile([P, TOK], F32, tag="nqbc")
        xTd = dramp.tile([Dm, TOK], F32, tag="xTd")

        # ================= attention (kv-resident scope) =================
        with tc.tile_pool(name="kv", bufs=1) as kp:
            kv_sb = [kp.tile([P, N], BF, tag=f"kv{dc}", name=f"kv{dc}")
                     for dc in range(DC)]
            q_sb = [kp.tile([P, TOK], BF, tag=f"q{dc}", name=f"q{dc}")
                    for dc in range(DC)]
            for dc in range(DC):
                nc.sync.dma_start(out=kv_sb[dc][:], in_=kvT[dc * P:(dc + 1) * P, :])
                nc.sync.dma_start(out=q_sb[dc][:], in_=qT[dc * P:(dc + 1) * P, :])

            # --- rms stats for kv (per key) and q (per token) ---
            with tc.tile_pool(name="statp", bufs=1, space="PSUM") as pn, \
                 tc.tile_pool(name="stats", bufs=2) as sp:
                sumsq = [pn.tile([1, MB], F32, tag=f"ss{nb}", name=f"ss{nb}")
                         for nb in range(NBN)]
                qss = pn.tile([1, TOK], F32, tag="qss")
                for dc in range(DC):
                    sq = sp.tile([P, N], BF, tag="sq")
                    nc.scalar.activation(sq[:], kv_sb[dc][:], AF.Square)
                    for nb in range(NBN):
                        nc.tensor.matmul(sumsq[nb][:], ones_bf[:],
                                         sq[:, nb * MB:(nb + 1) * MB],
                                         start=(dc == 0), stop=(dc == DC - 1))
                    sqq = sp.tile([P, TOK], BF, tag="sqq")
                    nc.scalar.activation(sqq[:], q_sb[dc][:], AF.Square)
                    nc.tensor.matmul(qss[:], ones_bf[:], sqq[:],
                                     start=(dc == 0), stop=(dc == DC - 1))
                ln_row = sp.tile([1, N], F32, tag="lnrow")
                for nb in range(NBN):
                    nc.scalar.activation(ln_row[0:1, nb * MB:(nb + 1) * MB],
                                         sumsq[nb][:], AF.Ln,
                                         scale=1.0 / Dm, bias=eps_t[0:1, :])
                lnq_row = sp.tile([1, TOK], F32, tag="lnqrow")
                nc.scalar.activation(lnq_row[:], qss[:], AF.Ln,
                                     scale=1.0 / Dm, bias=eps_t[0:1, :])
                nq_row = sp.tile([1, TOK], F32, tag="nqrow")
                nc.scalar.activation(nq_row[:], lnq_row[:], AF.Exp, scale=-0.5)
                drow = dramp.tile([1, N], F32, tag="dkv")
                nc.sync.dma_start(out=drow[:], in_=ln_row[:])
                lncol = sp.tile([P, NT], F32, tag="lncol")
                nc.sync.dma_start(out=lncol[:],
                                  in_=drow[:].rearrange("a (t p) -> (a p) t", p=P))
                nc.scalar.activation(nkv_col[:], lncol[:], AF.Exp,
                                     scale=-0.5, bias=lnsc_t[:])
                nc.scalar.activation(nkvV_col[:], lncol[:], AF.Exp, scale=-0.5)
                dq = dramp.tile([1, TOK], F32, tag="dq")
                nc.sync.dma_start(out=dq[:], in_=nq_row[:])
                nc.sync.dma_start(out=nq_bc[:], in_=dq[:].to_broadcast((P, TOK)))

            # --- head-pair group loop ---
            with tc.tile_pool(name="gw", bufs=1) as gw, \
                 tc.tile_pool(name="gk", bufs=1) as gk, \
                 tc.tile_pool(name="gv", bufs=1) as gv, \
                 tc.tile_pool(name="gs", bufs=2) as gs, \
                 tc.tile_pool(name="ep", bufs=4) as ep, \
                 tc.tile_pool(name="gp", bufs=2, space="PSUM") as pp:
                for g in range(NG):
                    wk_t = gw.tile([P, DC * GH], BF, tag="wk")
                    wq_t = gw.tile([P, DC * GH], BF, tag="wq")
                    wv_t = gw.tile([P, DC * GH], BF, tag="wv")
                    wkr_t = gw.tile([P, DC * P], BF, tag="wkr")
                    wqr_t = gw.tile([P, DC * P], BF, tag="wqr")
                    nc.sync.dma_start(out=wk_t[:],
                                      in_=wkp[g])
                    nc.sync.dma_start(out=wq_t[:],
                                      in_=wqp[g])
                    nc.sync.dma_start(out=wv_t[:],
                                      in_=wvp[g])
                    nc.sync.dma_start(out=wkr_t[:],
                                      in_=wkrp[g])
                    nc.sync.dma_start(out=wqr_t[:],
                                      in_=wqrp[g])
                    kt = [gk.tile([P, N], BF, tag=f"kt{h2}", name=f"kt{h2}")
                          for h2 in range(G2)]
                    krt = gk.tile([P, N], BF, tag="krt")
                    qt = [gk.tile([P, TOK], BF, tag=f"qt{h2}", name=f"qt{h2}")
                          for h2 in range(G2)]
                    qrt = gk.tile([P, TOK], BF, tag="qrt")
                    vt = [gv.tile([P, GH], BF, tag=f"vt{nt}", name=f"vt{nt}")
                          for nt in range(NT)]

                    # K_C projection
                    for h2 in range(G2):
                        for nb in range(NBN):
                            nbs = slice(nb * MB, (nb + 1) * MB)
                            ps = pp.tile([P, MB], F32, tag="proj")
                            for dc in range(DC):
                                nc.tensor.matmul(
                                    ps[:],
                                    wk_t[:, dc * GH + h2 * D_H:
                                         dc * GH + (h2 + 1) * D_H],
                                    kv_sb[dc][:, nbs],
                                    start=(dc == 0), stop=(dc == DC - 1))
                            nc.scalar.copy(kt[h2][:, nbs], ps[:])
                    # K_R projection (pair stacked on partitions) + rope
                    krr = gs.tile([P, N], BF, tag="krr")
                    for nb in range(NBN):
                        nbs = slice(nb * MB, (nb + 1) * MB)
                        ps = pp.tile([P, MB], F32, tag="proj")
                        for dc in range(DC):
                            nc.tensor.matmul(ps[:], wkr_t[:, dc * P:(dc + 1) * P],
                                             kv_sb[dc][:, nbs],
                                             start=(dc == 0), stop=(dc == DC - 1))
                        nc.scalar.copy(krr[:, nbs], ps[:])
                    for nb in range(NBN):
                        nbs = slice(nb * MB, (nb + 1) * MB)
                        rps = pp.tile([P, MB], F32, tag="proj")
                        nc.tensor.matmul(rps[:], rot_sb[:], krr[:, nbs],
                                         start=True, stop=True)
                        c_t = gs.tile([P, MB], BF, tag="rc")
                        nc.vector.tensor_mul(c_t[:], krr[:, nbs], cosK_sb[:, nbs])
                        s_t = gs.tile([P, MB], BF, tag="rs")
                        nc.vector.tensor_mul(s_t[:], rps[:], sinK_sb[:, nbs])
                        nc.vector.tensor_add(krt[:, nbs], c_t[:], s_t[:])
                    # V projection (keys on partitions), nkv-scaled evac
                    for nt in range(NT):
                        ps = pp.tile([P, GH], F32, tag="proj")
                        for dc in range(DC):
                            nc.tensor.matmul(ps[:],
                                             kv_sb[dc][:, nt * P:(nt + 1) * P],
                                             wv_t[:, dc * GH:(dc + 1) * GH],
                                             start=(dc == 0), stop=(dc == DC - 1))
                        nc.vector.tensor_scalar_mul(vt[nt][:], ps[:],
                                                    nkvV_col[:, nt:nt + 1])
                    # Q_C projection (nq folded at evac)
                    for h2 in range(G2):
                        ps = pp.tile([P, TOK], F32, tag="proj")
                        for dc in range(DC):
                            nc.tensor.matmul(
                                ps[:],
                                wq_t[:, dc * GH + h2 * D_H:
                                     dc * GH + (h2 + 1) * D_H],
                                q_sb[dc][:],
                                start=(dc == 0), stop=(dc == DC - 1))
                        nc.vector.tensor_mul(qt[h2][:], ps[:], nq_bc[:])
                    # Q_R projection + rope (nq folded pre-rope; rope commutes)
                    qrr = gs.tile([P, TOK], BF, tag="qrr")
                    ps = pp.tile([P, TOK], F32, tag="proj")
                    for dc in range(DC):
                        nc.tensor.matmul(ps[:], wqr_t[:, dc * P:(dc + 1) * P],
                                         q_sb[dc][:],
                                         start=(dc == 0), stop=(dc == DC - 1))
                    nc.vector.tensor_mul(qrr[:], ps[:], nq_bc[:])
                    rps = pp.tile([P, TOK], F32, tag="proj")
                    nc.tensor.matmul(rps[:], rot_sb[:], qrr[:],
                                     start=True, stop=True)
                    c_t = gs.tile([P, TOK], BF, tag="rc")
                    nc.vector.tensor_mul(c_t[:], qrr[:], cosQ_sb[:])
                    s_t = gs.tile([P, TOK], BF, tag="rs")
                    nc.vector.tensor_mul(s_t[:], rps[:], sinQ_sb[:])
                    nc.vector.tensor_add(qrt[:], c_t[:], s_t[:])

                    # attention per head in the pair
                    for h2 in range(G2):
                        h = g * G2 + h2
                        ro = h2 * D_R
                        u_ps = pp.tile([P, TOK], F32, tag="u")
                        sum_ps = pp.tile([1, TOK], F32, tag="sums")
                        for nt in range(NT):
                            s_ps = pp.tile([P, TOK], F32, tag="s")
                            nc.tensor.matmul(s_ps[:],
                                             kt[h2][:, nt * P:(nt + 1) * P],
                                             qt[h2][:], start=True, stop=False)
                            nc.tensor.matmul(
                                s_ps[:],
                                krt[ro:ro + D_R, nt * P:(nt + 1) * P],
                                qrt[ro:ro + D_R, :],
                                start=False, stop=True)
                            et = ep.tile([P, TOK], BF, tag="et")
                            nc.scalar.activation(et[:], s_ps[:], AF.Exp,
                                                 scale=nkv_col[:, nt:nt + 1])
                            nc.tensor.matmul(u_ps[:],
                                             vt[nt][:, h2 * D_H:(h2 + 1) * D_H],
                                             et[:], start=(nt == 0),
                                             stop=(nt == NT - 1))
                            nc.tensor.matmul(sum_ps[:], ones_bf[:], et[:],
                                             start=(nt == 0),
                                             stop=(nt == NT - 1))
                        rs_row = gs.tile([1, TOK], F32, tag="rsr")
                        nc.vector.reciprocal(rs_row[:], sum_ps[:])
                        drs = dramp.tile([1, TOK], F32, tag=f"drs{h}")
                        nc.sync.dma_start(out=drs[:], in_=rs_row[:])
                        rsb = gs.tile([P, TOK], F32, tag="rsb")
                        nc.sync.dma_start(out=rsb[:],
                                          in_=drs[:].to_broadcast((P, TOK)))
                        nc.vector.tensor_mul(ut[h][:], u_ps[:], rsb[:])

        # ================= W_O + residual -> xT (DRAM) =================
        with tc.tile_pool(name="wo", bufs=1) as wp, \
             tc.tile_pool(name="ws", bufs=2) as ws, \
             tc.tile_pool(name="wp2", bufs=4, space="PSUM") as pw:
            wo_sb = [wp.tile([P, D], BF, tag=f"wo{h}", name=f"wo{h}")
                     for h in range(N_H)]
            qr_sb = [wp.tile([P, TOK], F32, tag=f"qr{dt}", name=f"qr{dt}")
                     for dt in range(Dm // P)]
            for h in range(N_H):
                nc.sync.dma_start(out=wo_sb[h][:], in_=wop[h])
            for dt in range(Dm // P):
                nc.sync.dma_start(out=qr_sb[dt][:],
                                  in_=qres[dt * P:(dt + 1) * P, :])
            for dt in range(Dm // P):
                w_ps = pw.tile([P, TOK], F32, tag="wops")
                for h in range(N_H):
                    nc.tensor.matmul(w_ps[:],
                                     wo_sb[h][:, dt * P:(dt + 1) * P],
                                     ut[h][:], start=(h == 0),
                                     stop=(h == N_H - 1))
                xt_sb = ws.tile([P, TOK], F32, tag="xt")
                nc.vector.tensor_add(xt_sb[:], w_ps[:], qr_sb[dt][:])
                nc.sync.dma_start(out=xTd[dt * P:(dt + 1) * P, :], in_=xt_sb[:])

        # ================= FFN =================
        with tc.tile_pool(name="fs", bufs=3) as fs, \
             tc.tile_pool(name="fw", bufs=2) as fw, \
             tc.tile_pool(name="hp", bufs=1) as hp, \
             tc.tile_pool(name="fp", bufs=2, space="PSUM") as pf:
            # stats over xT
            ss_ps = pf.tile([1, TOK], F32, tag="ssx")
            for dt in range(DTT):
                xt_t = fs.tile([P, TOK], F32, tag="xt")
                nc.sync.dma_start(out=xt_t[:],
                                  in_=xTd[dt * P:(dt + 1) * P, :])
                sq = fs.tile([P, TOK], BF, tag="sq")
                nc.scalar.activation(sq[:], xt_t[:], AF.Square)
                nc.tensor.matmul(ss_ps[:], ones_bf[:], sq[:],
                                 start=(dt == 0), stop=(dt == DTT - 1))
            lnx = fs.tile([1, TOK], F32, tag="lnx")
            nc.scalar.activation(lnx[:], ss_ps[:], AF.Ln,
                                 scale=1.0 / Dm, bias=eps_t[0:1, :])
            nx_row = fs.tile([1, TOK], F32, tag="nxrow")
            nc.scalar.activation(nx_row[:], lnx[:], AF.Exp, scale=-0.5)
            dxn = dramp.tile([1, TOK], F32, tag="dxn")
            nc.sync.dma_start(out=dxn[:], in_=nx_row[:])
            nx_bc = fs.tile([P, TOK], F32, tag="nxb")
            nc.sync.dma_start(out=nx_bc[:], in_=dxn[:].to_broadcast((P, TOK)))
            xn_sb = [hp.tile([P, TOK], BF, tag=f"xn{dt}", name=f"xn{dt}")
                     for dt in range(DTT)]
            for dt in range(DTT):
                xt_t = fs.tile([P, TOK], F32, tag="xt")
                nc.sync.dma_start(out=xt_t[:],
                                  in_=xTd[dt * P:(dt + 1) * P, :])
                nc.vector.tensor_mul(xn_sb[dt][:], xt_t[:], nx_bc[:])
            # gate/up
            ht = [hp.tile([P, TOK], BF, tag=f"h{fc}", name=f"h{fc}")
                  for fc in range(FC)]
            for fc in range(FC):
                wg_t = fw.tile([P, DC * P], BF, tag="wg")
                wu_t = fw.tile([P, DC * P], BF, tag="wu")
                nc.sync.dma_start(out=wg_t[:],
                                  in_=wgp[fc])
                nc.sync.dma_start(out=wu_t[:],
                                  in_=wup[fc])
                g_ps = pf.tile([P, TOK], F32, tag="g")
                u_ps = pf.tile([P, TOK], F32, tag="gu")
                for dc in range(DC):
                    nc.tensor.matmul(g_ps[:], wg_t[:, dc * P:(dc + 1) * P],
                                     xn_sb[dc][:],
                                     start=(dc == 0), stop=(dc == DC - 1))
                    nc.tensor.matmul(u_ps[:], wu_t[:, dc * P:(dc + 1) * P],
                                     xn_sb[dc][:],
                                     start=(dc == 0), stop=(dc == DC - 1))
                hs = fs.tile([P, TOK], BF, tag="hs")
                nc.scalar.activation(hs[:], g_ps[:], AF.Silu)
                nc.vector.tensor_mul(ht[fc][:], hs[:], u_ps[:])
            # down + residual
            for dt in range(DTT):
                wd_t = fw.tile([P, FC * P], BF, tag="wd")
                nc.sync.dma_start(out=wd_t[:],
                                  in_=wdp[dt])
                d_ps = pf.tile([P, TOK], F32, tag="d")
                for fc in range(FC):
                    nc.tensor.matmul(d_ps[:], wd_t[:, fc * P:(fc + 1) * P],
                                     ht[fc][:],
                                     start=(fc == 0), stop=(fc == FC - 1))
                xt_t = fs.tile([P, TOK], F32, tag="xt")
                nc.sync.dma_start(out=xt_t[:],
                                  in_=xTd[dt * P:(dt + 1) * P, :])
                o_sb = fs.tile([P, TOK], F32, tag="o")
                nc.vector.tensor_add(o_sb[:], d_ps[:], xt_t[:])
                nc.sync.dma_start(out=yT[dt * P:(dt + 1) * P, :], in_=o_sb[:])

    prune_redundant_waits(nc, verbose=True)
    nc.compile()
    return nc


# --------------------------------------------------------------------------
# Host orchestration
# --------------------------------------------------------------------------
_prog_cache = {}


def _get(key, builder, *args):
    if key not in _prog_cache:
        _prog_cache[key] = builder(*args)
    return _prog_cache[key]


def _bf(x):
    return np.ascontiguousarray(np.asarray(x, dtype=np.float32)).astype(
        ml_dtypes.bfloat16)


def _rope_tables(S, dim):
    freqs = 1.0 / (THETA ** (np.arange(0, dim, 2, dtype=np.float32) / dim))
    f = np.arange(S, dtype=np.float32)[:, None] * freqs[None, :]
    cos = np.repeat(np.cos(f), 2, axis=-1).astype(np.float32)
    sin = np.repeat(np.sin(f), 2, axis=-1).astype(np.float32)
    return cos, sin


def _rot_lhsT(dim):
    rt = np.zeros((dim, dim), np.float32)
    for i in range(dim // 2):
        rt[2 * i + 1, 2 * i] = -1.0
        rt[2 * i, 2 * i + 1] = 1.0
    return rt


def _timed_run(nc, in_maps, reps=10):
    """Execute on all cores with device-resident inputs; time warm reps.

    Returns (results_list, best_exec_seconds). Mimics
    bass2jax.run_bass_via_pjrt's multi-core path but keeps inputs on
    device so the timed region is pure NEFF execution + dispatch.
    """
    import time as _time
    import jax
    from jax.sharding import Mesh, PartitionSpec, NamedSharding
    from jax.experimental.shard_map import shard_map
    from concourse import bass2jax as b2j
    from concourse import mybir as _mb

    b2j.install_neuronx_cc_hook()
    n_cores = len(in_maps)
    in_names, out_names, out_avals, zero_outs = [], [], [], []
    for alloc in nc.m.functions[0].allocations:
        if not isinstance(alloc, _mb.MemoryLocationSet):
            continue
        name = alloc.memorylocations[0].name
        pid_name = nc.partition_id_tensor.name if nc.partition_id_tensor else None
        if alloc.kind == "ExternalInput":
            if name != pid_name:
                in_names.append(name)
        elif alloc.kind == "ExternalOutput":
            out_names.append(name)
            shape = tuple(alloc.tensor_shape)
            dtype = _mb.dt.np(alloc.dtype)
            out_avals.append(jax.core.ShapedArray(shape, dtype))
            zero_outs.append(np.zeros(shape, dtype))
    n_params = len(in_names)
    n_outs = len(out_avals)
    all_names = list(in_names) + list(out_names)
    if nc.partition_id_tensor is not None:
        all_names.append(nc.partition_id_tensor.name)

    def _body(*args):
        operands = list(args)
        if nc.partition_id_tensor is not None:
            operands.append(b2j.partition_id_tensor())
        outs = b2j._bass_exec_p.bind(
            *operands, out_avals=tuple(out_avals), in_names=tuple(all_names),
            out_names=tuple(out_names), lowering_input_output_aliases=(),
            sim_require_finite=True, sim_require_nnan=True, nc=nc)
        return tuple(outs)

    devices = jax.devices()[:n_cores]
    mesh = Mesh(np.asarray(devices), ("core",))
    donate = tuple(range(n_params, n_params + n_outs))
    sharded = jax.jit(
        shard_map(_body, mesh=mesh,
                  in_specs=(PartitionSpec("core"),) * (n_params + n_outs),
                  out_specs=(PartitionSpec("core"),) * n_outs,
                  check_rep=False),
        donate_argnums=donate, keep_unused=True)
    sh = NamedSharding(mesh, PartitionSpec("core"))
    dev_in = [jax.device_put(
        np.concatenate([np.asarray(in_maps[c][n]) for c in range(n_cores)], axis=0), sh)
        for n in in_names]
    # warmup + correctness outputs
    dz = [jax.device_put(
        np.zeros((n_cores * z.shape[0], *z.shape[1:]), z.dtype), sh)
        for z in zero_outs]
    jax.block_until_ready(dz)
    outs = sharded(*dev_in, *dz)
    jax.block_until_ready(outs)
    # pipelined timing: amortize per-call dispatch overhead over reps
    zsets = [[jax.device_put(
        np.zeros((n_cores * z.shape[0], *z.shape[1:]), z.dtype), sh)
        for z in zero_outs] for _ in range(reps)]
    jax.block_until_ready(zsets)
    t0 = _time.perf_counter()
    last = None
    for k in range(reps):
        last = sharded(*dev_in, *zsets[k])
    jax.block_until_ready(last)
    total = _time.perf_counter() - t0
    best = total / reps
    results = [
        {name: np.asarray(outs[i]).reshape(n_cores, *out_avals[i].shape)[c]
         for i, name in enumerate(out_names)}
        for c in range(n_cores)]
    return results, best


_last_exec_ns = []


class _Res:
    def __init__(self, results):
        self.results = results


def _run(nc, in_maps, trace=False):
    if trace:
        results, secs = _timed_run(nc, in_maps)
        _last_exec_ns.append(int(secs * 1e9))
        return _Res(results)
    res = run_bass_kernel_spmd(nc, in_maps, list(range(len(in_maps))))
    _last_exec_ns.append(res.exec_time_ns)
    return res


def kernel(query, key_value, g_q, g_kv, g_ffn, w_qc, w_kc, w_qr, w_kr, w_v,
           w_o, w_gate, w_up, w_down, _trace=False):
    query = np.asarray(query, np.float32)
    key_value = np.asarray(key_value, np.float32)
    Bq, Mq, _ = query.shape
    Nq = key_value.shape[1]
    NG = N_H // G2
    DC = D // P
    GH = G2 * D_H
    FC = FFN // P
    DTT = D // P
    TQ = Bq * Mq // (NCORES * TOK) * 4 // 1  # quarters per batch
    TQ = NCORES // Bq                        # 4 token quarters per batch

    g_q = np.asarray(g_q, np.float32)[:, None]
    g_kv = np.asarray(g_kv, np.float32)[:, None]
    g_ffn = np.asarray(g_ffn, np.float32)[:, None]
    wqc = np.asarray(w_qc, np.float32) * g_q
    wqr_f = np.asarray(w_qr, np.float32) * g_q
    wkc = np.asarray(w_kc, np.float32) * g_kv
    wkr_f = np.asarray(w_kr, np.float32) * g_kv
    wv_f = np.asarray(w_v, np.float32) * g_kv
    wo_f = np.asarray(w_o, np.float32)
    wgate = np.asarray(w_gate, np.float32) * g_ffn
    wup = np.asarray(w_up, np.float32) * g_ffn
    wdown = np.asarray(w_down, np.float32)

    # packed weight layouts (contiguous per-group / per-chunk DMAs)
    wkp = _bf(wkc).reshape(DC, P, NG, GH).transpose(2, 1, 0, 3).reshape(NG, P, DC * GH).copy()
    wqp = _bf(wqc).reshape(DC, P, NG, GH).transpose(2, 1, 0, 3).reshape(NG, P, DC * GH).copy()
    wvp = _bf(wv_f).reshape(DC, P, NG, GH).transpose(2, 1, 0, 3).reshape(NG, P, DC * GH).copy()
    wkrp = _bf(wkr_f).reshape(DC, P, NG, P).transpose(2, 1, 0, 3).reshape(NG, P, DC * P).copy()
    wqrp = _bf(wqr_f).reshape(DC, P, NG, P).transpose(2, 1, 0, 3).reshape(NG, P, DC * P).copy()
    wop = _bf(wo_f).reshape(N_H, P, D).copy()
    wgp_ = _bf(wgate).reshape(DC, P, FC, P).transpose(2, 1, 0, 3).reshape(FC, P, DC * P).copy()
    wup_ = _bf(wup).reshape(DC, P, FC, P).transpose(2, 1, 0, 3).reshape(FC, P, DC * P).copy()
    wdp_ = _bf(wdown).reshape(FC, P, DTT, P).transpose(2, 1, 0, 3).reshape(DTT, P, FC * P).copy()

    cos, sin = _rope_tables(max(Mq, Nq), D_R)
    cosK = np.ascontiguousarray(np.vstack([cos[:Nq].T] * G2)).astype(np.float16)
    sinK = np.ascontiguousarray(np.vstack([sin[:Nq].T] * G2)).astype(np.float16)
    rot2 = _bf(np.kron(np.eye(G2, dtype=np.float32), _rot_lhsT(D_R)))

    kvT_b = [_bf(key_value[b].T) for b in range(Bq)]

    del _last_exec_ns[:]
    nc = _get(("fused", Nq, D), build_fused, Nq, D)
    in_maps = []
    for c in range(NCORES):
        b, tq = c // TQ, c % TQ
        sl = slice(tq * TOK, (tq + 1) * TOK)
        qs = query[b, sl, :]
        in_maps.append({
            "kvT": kvT_b[b],
            "qT": _bf(qs.T),
            "qres": np.ascontiguousarray(qs.T),
            "wkp": wkp, "wqp": wqp, "wvp": wvp,
            "wkrp": wkrp, "wqrp": wqrp, "wop": wop,
            "wgp": wgp_, "wup": wup_, "wdp": wdp_,
            "cosK": cosK, "sinK": sinK,
            "cosQ": np.ascontiguousarray(
                np.vstack([cos[sl].T] * G2)).astype(np.float16),
            "sinQ": np.ascontiguousarray(
                np.vstack([sin[sl].T] * G2)).astype(np.float16),
            "rot2": rot2,
        })
    res = _run(nc, in_maps, trace=_trace)

    y = np.empty((Bq, Mq, D), np.float32)
    for c in range(NCORES):
        b, tq = c // TQ, c % TQ
        y[b, tq * TOK:(tq + 1) * TOK, :] = res.results[c]["yT"].T
    return y


# revision 6
# speedup vs baseline: 2.0217x; 1.0002x over previous
"""Trainium2 Bass kernel for nn_AttentionBlock (MLA-style attention + SwiGLU FFN).

Self-contained: takes FULL inputs, shards across 8 NeuronCores internally,
returns FULL output.

Sharding (single fused launch, zero collectives):
  Each core owns 512 query tokens (batch b = c//4, quarter tq = c%4) and
  computes the WHOLE block for them: KV projections for its batch (all 16
  heads, duplicated across the 4 cores sharing a batch - cheaper than any
  cross-core reduction here), attention in head-pair groups, W_O + residual,
  RMS norm, token-local FFN, final residual. Output is the core's yT
  [D, 512] f32 slice; host transposes and concatenates.
All matmuls run in bf16 with fp32 PSUM accumulation, moving-operand free
size 512. Softmax/norm statistics are fp32 (rsqrt via the Ln+Exp table so
the whole kernel needs one activation-table switch, to Silu, at the FFN).
"""
import sys
sys.path.insert(0, '/opt/trn_rl_repo')

import math
import numpy as np
import ml_dtypes

from concourse import bass, bacc, mybir, tile
from concourse.bass_utils import run_bass_kernel_spmd

# ---- inlined wait pruner (kernel.py must be self-contained) ----
import bisect


def _is_dma(inst):
    return type(inst).__name__ in (
        "InstDMACopy", "InstDmaTranspose", "InstDmaTransposeAnt",
        "InstTensorCopyDma", "InstTensorReduceDma")


def prune_redundant_waits(nc, verbose=False):
    insts = []
    for f in nc.m.functions:
        for blk in f.blocks:
            insts.extend(blk.instructions)

    poisoned = set()
    running = {}
    producers = {}   # sem -> ([values], [idx])
    VC = [None] * len(insts)
    chain_vc = {}    # engine -> completion vc of last instruction
    chain_prev = [None] * len(insts)   # vc inherited from chain (pre-wait)

    def producer_at_least(sem, v):
        if sem in poisoned or sem not in producers:
            return None
        vals, idxs = producers[sem]
        i = bisect.bisect_left(vals, v)
        if i == len(vals):
            return None
        return vals[i], idxs[i]

    def merge(dst, src):
        for s, v in src.items():
            if dst.get(s, -1) < v:
                dst[s] = v

    for idx, inst in enumerate(insts):
        si = inst.sync_info
        is_dma = _is_dma(inst)
        ekey = getattr(inst, "engine", None)
        if is_dma:
            vc = {}
        else:
            vc = dict(chain_vc.get(ekey, {}))
        chain_prev[idx] = dict(vc)
        if si is not None:
            for w in si.on_wait:
                if w.wait_mode != "sem-ge-imm" or w.id in poisoned:
                    continue
                p = producer_at_least(w.id, w.wait_value)
                if p is not None:
                    merge(vc, VC[p[1]])
                    if vc.get(w.id, -1) < p[0]:
                        vc[w.id] = p[0]
                else:
                    if vc.get(w.id, -1) < w.wait_value:
                        vc[w.id] = w.wait_value
            for u in si.on_update:
                if u.update_mode in ("sem-inc", "sem-add-imm"):
                    nv = running.get(u.id, 0) + u.update_value
                    running[u.id] = nv
                    producers.setdefault(u.id, ([], []))
                    producers[u.id][0].append(nv)
                    producers[u.id][1].append(idx)
                    if vc.get(u.id, -1) < nv:
                        vc[u.id] = nv
                else:
                    poisoned.add(u.id)
        VC[idx] = vc
        if not is_dma:
            chain_vc[ekey] = vc

    # pass 2: prune
    n_pruned = 0
    for idx, inst in enumerate(insts):
        si = inst.sync_info
        if si is None or len(si.on_wait) < 2:
            continue
        waits = list(si.on_wait)
        kept = list(waits)
        changed = True
        while changed and len(kept) > 1:
            changed = False
            for w in kept:
                if w.wait_mode != "sem-ge-imm" or w.id in poisoned:
                    continue
                cover = dict(chain_prev[idx])
                ok_others = True
                for o in kept:
                    if o is w:
                        continue
                    if o.wait_mode != "sem-ge-imm" or o.id in poisoned:
                        continue
                    p = producer_at_least(o.id, o.wait_value)
                    if p is not None:
                        merge(cover, VC[p[1]])
                if cover.get(w.id, -1) >= w.wait_value:
                    kept.remove(w)
                    n_pruned += 1
                    changed = True
                    break
        if len(kept) != len(waits):
            import concourse.mybir as mybir
            inst.sync_info = mybir.SyncInfo(on_wait=kept, on_update=list(si.on_update))
    if verbose:
        print(f"wait_prune: removed {n_pruned} waits over {len(insts)} insts")
    return n_pruned

# ---- end wait pruner ----


BF = mybir.dt.bfloat16
F16 = mybir.dt.float16
F32 = mybir.dt.float32
AF = mybir.ActivationFunctionType

D = 2048
N_H = 16
D_H = 128
D_R = 64
FFN = 8192
THETA = 1000000.0
EPS = 1e-6
SCALE = 1.0 / math.sqrt(D_H + D_R)
NCORES = 8
P = 128
MB = 512
TOK = 512            # query tokens per core
G2 = 2               # heads per group (one rope pair-tile)


# --------------------------------------------------------------------------
# Fused single-launch kernel builder
# --------------------------------------------------------------------------
def build_fused(N=2048, Dm=D):
    NG = N_H // G2       # 8 head groups
    DC = Dm // P         # 16 contraction chunks
    NT = N // P          # 16 key tiles
    NBN = N // MB        # 4 key blocks of 512
    DTT = Dm // P        # 16 output d-tiles
    FC = FFN // P        # 64 ffn chunks
    GH = G2 * D_H        # 256 cols per group (K_C/Q_C/V)
    ln_scale_bias = float(math.log(SCALE))

    nc = bacc.Bacc()
    kvT = nc.dram_tensor("kvT", [Dm, N], BF, kind="ExternalInput")
    qT = nc.dram_tensor("qT", [Dm, TOK], BF, kind="ExternalInput")
    qres = nc.dram_tensor("qres", [Dm, TOK], F32, kind="ExternalInput")
    wkp = nc.dram_tensor("wkp", [NG, P, DC * GH], BF, kind="ExternalInput")
    wqp = nc.dram_tensor("wqp", [NG, P, DC * GH], BF, kind="ExternalInput")
    wvp = nc.dram_tensor("wvp", [NG, P, DC * GH], BF, kind="ExternalInput")
    wkrp = nc.dram_tensor("wkrp", [NG, P, DC * P], BF, kind="ExternalInput")
    wqrp = nc.dram_tensor("wqrp", [NG, P, DC * P], BF, kind="ExternalInput")
    wop = nc.dram_tensor("wop", [N_H, P, Dm], BF, kind="ExternalInput")
    wgp = nc.dram_tensor("wgp", [FC, P, DC * P], BF, kind="ExternalInput")
    wup = nc.dram_tensor("wup", [FC, P, DC * P], BF, kind="ExternalInput")
    wdp = nc.dram_tensor("wdp", [DTT, P, FC * P], BF, kind="ExternalInput")
    cosK = nc.dram_tensor("cosK", [P, N], F16, kind="ExternalInput")
    sinK = nc.dram_tensor("sinK", [P, N], F16, kind="ExternalInput")
    cosQ = nc.dram_tensor("cosQ", [P, TOK], F16, kind="ExternalInput")
    sinQ = nc.dram_tensor("sinQ", [P, TOK], F16, kind="ExternalInput")
    rot2 = nc.dram_tensor("rot2", [P, P], BF, kind="ExternalInput")
    yT = nc.dram_tensor("yT", [Dm, TOK], F32, kind="ExternalOutput")

    with tile.TileContext(nc) as tc:
      with tc.tile_pool(name="const", bufs=1) as cp, \
           tc.tile_pool(name="dram", bufs=1, space="DRAM") as dramp, \
           tc.tile_pool(name="outer", bufs=1) as op:
        ones_bf = cp.tile([P, 1], BF, tag="ones")
        nc.vector.memset(ones_bf[:], 1.0)
        eps_t = cp.tile([P, 1], F32, tag="eps")
        nc.vector.memset(eps_t[:], EPS)
        lnsc_t = cp.tile([P, 1], F32, tag="lnsc")
        nc.vector.memset(lnsc_t[:], ln_scale_bias)
        cosK_sb = cp.tile([P, N], F16, tag="cosK")
        sinK_sb = cp.tile([P, N], F16, tag="sinK")
        cosQ_sb = cp.tile([P, TOK], F16, tag="cosQ")
        sinQ_sb = cp.tile([P, TOK], F16, tag="sinQ")
        rot_sb = cp.tile([P, P], BF, tag="rot")
        nc.sync.dma_start(out=cosK_sb[:], in_=cosK[:])
        nc.sync.dma_start(out=sinK_sb[:], in_=sinK[:])
        nc.sync.dma_start(out=cosQ_sb[:], in_=cosQ[:])
        nc.sync.dma_start(out=sinQ_sb[:], in_=sinQ[:])
        nc.sync.dma_start(out=rot_sb[:], in_=rot2[:])

        ut = [op.tile([P, TOK], BF, tag=f"ut{h}", name=f"ut{h}")
              for h in range(N_H)]
        nkv_col = op.tile([P, NT], F32, tag="nkvc")
        nkvV_col = op.tile([P, NT], F32, tag="nkvvc")
        nq_bc = op.tile([P, TOK], F32, tag="nqbc")
        xTd = dramp.tile([Dm, TOK], F32, tag="xTd")

        # ================= attention (kv-resident scope) =================
        with tc.tile_pool(name="kv", bufs=1) as kp:
            kv_sb = [kp.tile([P, N], BF, tag=f"kv{dc}", name=f"kv{dc}")
                     for dc in range(DC)]
            q_sb = [kp.tile([P, TOK], BF, tag=f"q{dc}", name=f"q{dc}")
                    for dc in range(DC)]
            for dc in range(DC):
                nc.sync.dma_start(out=kv_sb[dc][:], in_=kvT[dc * P:(dc + 1) * P, :])
                nc.sync.dma_start(out=q_sb[dc][:], in_=qT[dc * P:(dc + 1) * P, :])

            # --- rms stats for kv (per key) and q (per token) ---
            with tc.tile_pool(name="statp", bufs=1, space="PSUM") as pn, \
                 tc.tile_pool(name="stats", bufs=2) as sp:
                sumsq = [pn.tile([1, MB], F32, tag=f"ss{nb}", name=f"ss{nb}")
                         for nb in range(NBN)]
                qss = pn.tile([1, TOK], F32, tag="qss")
                for dc in range(DC):
                    sq = sp.tile([P, N], BF, tag="sq")
                    nc.scalar.activation(sq[:], kv_sb[dc][:], AF.Square)
                    for nb in range(NBN):
                        nc.tensor.matmul(sumsq[nb][:], ones_bf[:],
                                         sq[:, nb * MB:(nb + 1) * MB],
                                         start=(dc == 0), stop=(dc == DC - 1))
                    sqq = sp.tile([P, TOK], BF, tag="sqq")
                    nc.scalar.activation(sqq[:], q_sb[dc][:], AF.Square)
                    nc.tensor.matmul(qss[:], ones_bf[:], sqq[:],
                                     start=(dc == 0), stop=(dc == DC - 1))
                ln_row = sp.tile([1, N], F32, tag="lnrow")
                for nb in range(NBN):
                    nc.scalar.activation(ln_row[0:1, nb * MB:(nb + 1) * MB],
                                         sumsq[nb][:], AF.Ln,
                                         scale=1.0 / Dm, bias=eps_t[0:1, :])
                lnq_row = sp.tile([1, TOK], F32, tag="lnqrow")
                nc.scalar.activation(lnq_row[:], qss[:], AF.Ln,
                                     scale=1.0 / Dm, bias=eps_t[0:1, :])
                nq_row = sp.tile([1, TOK], F32, tag="nqrow")
                nc.scalar.activation(nq_row[:], lnq_row[:], AF.Exp, scale=-0.5)
                drow = dramp.tile([1, N], F32, tag="dkv")
                nc.sync.dma_start(out=drow[:], in_=ln_row[:])
                lncol = sp.tile([P, NT], F32, tag="lncol")
                nc.sync.dma_start(out=lncol[:],
                                  in_=drow[:].rearrange("a (t p) -> (a p) t", p=P))
                nc.scalar.activation(nkv_col[:], lncol[:], AF.Exp,
                                     scale=-0.5, bias=lnsc_t[:])
                nc.scalar.activation(nkvV_col[:], lncol[:], AF.Exp, scale=-0.5)
                dq = dramp.tile([1, TOK], F32, tag="dq")
                nc.sync.dma_start(out=dq[:], in_=nq_row[:])
                nc.sync.dma_start(out=nq_bc[:], in_=dq[:].to_broadcast((P, TOK)))

            # --- head-pair group loop ---
            with tc.tile_pool(name="gw", bufs=1) as gw, \
                 tc.tile_pool(name="gk", bufs=1) as gk, \
                 tc.tile_pool(name="gv", bufs=1) as gv, \
                 tc.tile_pool(name="gs", bufs=2) as gs, \
                 tc.tile_pool(name="ep", bufs=4) as ep, \
                 tc.tile_pool(name="gp", bufs=2, space="PSUM") as pp:
                for g in range(NG):
                    wk_t = gw.tile([P, DC * GH], BF, tag="wk")
                    wq_t = gw.tile([P, DC * GH], BF, tag="wq")
                    wv_t = gw.tile([P, DC * GH], BF, tag="wv")
                    wkr_t = gw.tile([P, DC * P], BF, tag="wkr")
                    wqr_t = gw.tile([P, DC * P], BF, tag="wqr")
                    nc.sync.dma_start(out=wk_t[:],
                                      in_=wkp[g])
                    nc.sync.dma_start(out=wq_t[:],
                                      in_=wqp[g])
                    nc.sync.dma_start(out=wv_t[:],
                                      in_=wvp[g])
                    nc.sync.dma_start(out=wkr_t[:],
                                      in_=wkrp[g])
                    nc.sync.dma_start(out=wqr_t[:],
                                      in_=wqrp[g])
                    kt = [gk.tile([P, N], BF, tag=f"kt{h2}", name=f"kt{h2}")
                          for h2 in range(G2)]
                    krt = gk.tile([P, N], BF, tag="krt")
                    qt = [gk.tile([P, TOK], BF, tag=f"qt{h2}", name=f"qt{h2}")
                          for h2 in range(G2)]
                    qrt = gk.tile([P, TOK], BF, tag="qrt")
                    vt = [gv.tile([P, GH], BF, tag=f"vt{nt}", name=f"vt{nt}")
                          for nt in range(NT)]

                    # K_C projection
                    for h2 in range(G2):
                        for nb in range(NBN):
                            nbs = slice(nb * MB, (nb + 1) * MB)
                            ps = pp.tile([P, MB], F32, tag="proj")
                            for dc in range(DC):
                                nc.tensor.matmul(
                                    ps[:],
                                    wk_t[:, dc * GH + h2 * D_H:
                                         dc * GH + (h2 + 1) * D_H],
                                    kv_sb[dc][:, nbs],
                                    start=(dc == 0), stop=(dc == DC - 1))
                            nc.scalar.copy(kt[h2][:, nbs], ps[:])
                    # K_R projection (pair stacked on partitions) + rope
                    krr = gs.tile([P, N], BF, tag="krr")
                    for nb in range(NBN):
                        nbs = slice(nb * MB, (nb + 1) * MB)
                        ps = pp.tile([P, MB], F32, tag="proj")
                        for dc in range(DC):
                            nc.tensor.matmul(ps[:], wkr_t[:, dc * P:(dc + 1) * P],
                                             kv_sb[dc][:, nbs],
                                             start=(dc == 0), stop=(dc == DC - 1))
                        nc.scalar.copy(krr[:, nbs], ps[:])
                    for nb in range(NBN):
                        nbs = slice(nb * MB, (nb + 1) * MB)
                        rps = pp.tile([P, MB], F32, tag="proj")
                        nc.tensor.matmul(rps[:], rot_sb[:], krr[:, nbs],
                                         start=True, stop=True)
                        c_t = gs.tile([P, MB], BF, tag="rc")
                        nc.vector.tensor_mul(c_t[:], krr[:, nbs], cosK_sb[:, nbs])
                        s_t = gs.tile([P, MB], BF, tag="rs")
                        nc.vector.tensor_mul(s_t[:], rps[:], sinK_sb[:, nbs])
                        nc.vector.tensor_add(krt[:, nbs], c_t[:], s_t[:])
                    # V projection (keys on partitions), nkv-scaled evac
                    for nt in range(NT):
                        ps = pp.tile([P, GH], F32, tag="proj")
                        for dc in range(DC):
                            nc.tensor.matmul(ps[:],
                                             kv_sb[dc][:, nt * P:(nt + 1) * P],
                                             wv_t[:, dc * GH:(dc + 1) * GH],
                                             start=(dc == 0), stop=(dc == DC - 1))
                        nc.vector.tensor_scalar_mul(vt[nt][:], ps[:],
                                                    nkvV_col[:, nt:nt + 1])
                    # Q_C projection (nq folded at evac)
                    for h2 in range(G2):
                        ps = pp.tile([P, TOK], F32, tag="proj")
                        for dc in range(DC):
                            nc.tensor.matmul(
                                ps[:],
                                wq_t[:, dc * GH + h2 * D_H:
                                     dc * GH + (h2 + 1) * D_H],
                                q_sb[dc][:],
                                start=(dc == 0), stop=(dc == DC - 1))
                        nc.vector.tensor_mul(qt[h2][:], ps[:], nq_bc[:])
                    # Q_R projection + rope (nq folded pre-rope; rope commutes)
                    qrr = gs.tile([P, TOK], BF, tag="qrr")
                    ps = pp.tile([P, TOK], F32, tag="proj")
                    for dc in range(DC):
                        nc.tensor.matmul(ps[:], wqr_t[:, dc * P:(dc + 1) * P],
                                         q_sb[dc][:],
                                         start=(dc == 0), stop=(dc == DC - 1))
                    nc.vector.tensor_mul(qrr[:], ps[:], nq_bc[:])
                    rps = pp.tile([P, TOK], F32, tag="proj")
                    nc.tensor.matmul(rps[:], rot_sb[:], qrr[:],
                                     start=True, stop=True)
                    c_t = gs.tile([P, TOK], BF, tag="rc")
                    nc.vector.tensor_mul(c_t[:], qrr[:], cosQ_sb[:])
                    s_t = gs.tile([P, TOK], BF, tag="rs")
                    nc.vector.tensor_mul(s_t[:], rps[:], sinQ_sb[:])
                    nc.vector.tensor_add(qrt[:], c_t[:], s_t[:])

                    # attention per head in the pair
                    for h2 in range(G2):
                        h = g * G2 + h2
                        ro = h2 * D_R
                        u_ps = pp.tile([P, TOK], F32, tag="u")
                        sum_ps = pp.tile([1, TOK], F32, tag="sums")
                        for nt in range(NT):
                            s_ps = pp.tile([P, TOK], F32, tag="s")
                            nc.tensor.matmul(s_ps[:],
                                             kt[h2][:, nt * P:(nt + 1) * P],
                                             qt[h2][:], start=True, stop=False)
                            nc.tensor.matmul(
                                s_ps[:],
                                krt[ro:ro + D_R, nt * P:(nt + 1) * P],
                                qrt[ro:ro + D_R, :],
                                start=False, stop=True)
                            et = ep.tile([P, TOK], BF, tag="et")
                            nc.scalar.activation(et[:], s_ps[:], AF.Exp,
                                                 scale=nkv_col[:, nt:nt + 1])
                            nc.tensor.matmul(u_ps[:],
                                             vt[nt][:, h2 * D_H:(h2 + 1) * D_H],
                                             et[:], start=(nt == 0),
                                             stop=(nt == NT - 1))
                            nc.tensor.matmul(sum_ps[:], ones_bf[:], et[:],
                                             start=(nt == 0),
                                             stop=(nt == NT - 1))
                        rs_row = gs.tile([1, TOK], F32, tag="rsr")
                        nc.vector.reciprocal(rs_row[:], sum_ps[:])
                        drs = dramp.tile([1, TOK], F32, tag=f"drs{h}")
                        nc.sync.dma_start(out=drs[:], in_=rs_row[:])
                        rsb = gs.tile([P, TOK], F32, tag="rsb")
                        nc.sync.dma_start(out=rsb[:],
                                          in_=drs[:].to_broadcast((P, TOK)))
                        nc.vector.tensor_mul(ut[h][:], u_ps[:], rsb[:])

        # ================= W_O + residual -> xT (DRAM) =================
        with tc.tile_pool(name="wo", bufs=1) as wp, \
             tc.tile_pool(name="ws", bufs=2) as ws, \
             tc.tile_pool(name="wp2", bufs=4, space="PSUM") as pw:
            wo_sb = [wp.tile([P, D], BF, tag=f"wo{h}", name=f"wo{h}")
                     for h in range(N_H)]
            qr_sb = [wp.tile([P, TOK], F32, tag=f"qr{dt}", name=f"qr{dt}")
                     for dt in range(Dm // P)]
            for h in range(N_H):
                nc.sync.dma_start(out=wo_sb[h][:], in_=wop[h])
            for dt in range(Dm // P):
                nc.sync.dma_start(out=qr_sb[dt][:],
                                  in_=qres[dt * P:(dt + 1) * P, :])
            for dt in range(Dm // P):
                w_ps = pw.tile([P, TOK], F32, tag="wops")
                for h in range(N_H):
                    nc.tensor.matmul(w_ps[:],
                                     wo_sb[h][:, dt * P:(dt + 1) * P],
                                     ut[h][:], start=(h == 0),
                                     stop=(h == N_H - 1))
                xt_sb = ws.tile([P, TOK], F32, tag="xt")
                nc.vector.tensor_add(xt_sb[:], w_ps[:], qr_sb[dt][:])
                nc.sync.dma_start(out=xTd[dt * P:(dt + 1) * P, :], in_=xt_sb[:])

        # ================= FFN =================
        with tc.tile_pool(name="fs", bufs=3) as fs, \
             tc.tile_pool(name="fw", bufs=2) as fw, \
             tc.tile_pool(name="hp", bufs=1) as hp, \
             tc.tile_pool(name="fp", bufs=2, space="PSUM") as pf:
            # stats over xT
            ss_ps = pf.tile([1, TOK], F32, tag="ssx")
            for dt in range(DTT):
                xt_t = fs.tile([P, TOK], F32, tag="xt")
                nc.sync.dma_start(out=xt_t[:],
                                  in_=xTd[dt * P:(dt + 1) * P, :])
                sq = fs.tile([P, TOK], BF, tag="sq")
                nc.scalar.activation(sq[:], xt_t[:], AF.Square)
                nc.tensor.matmul(ss_ps[:], ones_bf[:], sq[:],
                                 start=(dt == 0), stop=(dt == DTT - 1))
            lnx = fs.tile([1, TOK], F32, tag="lnx")
            nc.scalar.activation(lnx[:], ss_ps[:], AF.Ln,
                                 scale=1.0 / Dm, bias=eps_t[0:1, :])
            nx_row = fs.tile([1, TOK], F32, tag="nxrow")
            nc.scalar.activation(nx_row[:], lnx[:], AF.Exp, scale=-0.5)
            dxn = dramp.tile([1, TOK], F32, tag="dxn")
            nc.sync.dma_start(out=dxn[:], in_=nx_row[:])
            nx_bc = fs.tile([P, TOK], F32, tag="nxb")
            nc.sync.dma_start(out=nx_bc[:], in_=dxn[:].to_broadcast((P, TOK)))
            xn_sb = [hp.tile([P, TOK], BF, tag=f"xn{dt}", name=f"xn{dt}")
                     for dt in range(DTT)]
            for dt in range(DTT):
                xt_t = fs.tile([P, TOK], F32, tag="xt")
                nc.sync.dma_start(out=xt_t[:],
                                  in_=xTd[dt * P:(dt + 1) * P, :])
                nc.vector.tensor_mul(xn_sb[dt][:], xt_t[:], nx_bc[:])
            # gate/up
            ht = [hp.tile([P, TOK], BF, tag=f"h{fc}", name=f"h{fc}")
                  for fc in range(FC)]
            for fc in range(FC):
                wg_t = fw.tile([P, DC * P], BF, tag="wg")
                wu_t = fw.tile([P, DC * P], BF, tag="wu")
                nc.sync.dma_start(out=wg_t[:],
                                  in_=wgp[fc])
                nc.sync.dma_start(out=wu_t[:],
                                  in_=wup[fc])
                g_ps = pf.tile([P, TOK], F32, tag="g")
                u_ps = pf.tile([P, TOK], F32, tag="gu")
                for dc in range(DC):
                    nc.tensor.matmul(g_ps[:], wg_t[:, dc * P:(dc + 1) * P],
                                     xn_sb[dc][:],
                                     start=(dc == 0), stop=(dc == DC - 1))
                    nc.tensor.matmul(u_ps[:], wu_t[:, dc * P:(dc + 1) * P],
                                     xn_sb[dc][:],
                                     start=(dc == 0), stop=(dc == DC - 1))
                hs = fs.tile([P, TOK], BF, tag="hs")
                nc.scalar.activation(hs[:], g_ps[:], AF.Silu)
                nc.vector.tensor_mul(ht[fc][:], hs[:], u_ps[:])
            # down + residual
            for dt in range(DTT):
                wd_t = fw.tile([P, FC * P], BF, tag="wd")
                nc.sync.dma_start(out=wd_t[:],
                                  in_=wdp[dt])
                d_ps = pf.tile([P, TOK], F32, tag="d")
                for fc in range(FC):
                    nc.tensor.matmul(d_ps[:], wd_t[:, fc * P:(fc + 1) * P],
                                     ht[fc][:],
                                     start=(fc == 0), stop=(fc == FC - 1))
                xt_t = fs.tile([P, TOK], F32, tag="xt")
                nc.sync.dma_start(out=xt_t[:],
                                  in_=xTd[dt * P:(dt + 1) * P, :])
                o_sb = fs.tile([P, TOK], F32, tag="o")
                nc.vector.tensor_add(o_sb[:], d_ps[:], xt_t[:])
                nc.sync.dma_start(out=yT[dt * P:(dt + 1) * P, :], in_=o_sb[:])

    prune_redundant_waits(nc, verbose=True)
    nc.compile()
    return nc


# --------------------------------------------------------------------------
# Host orchestration
# --------------------------------------------------------------------------
_prog_cache = {}


def _get(key, builder, *args):
    if key not in _prog_cache:
        _prog_cache[key] = builder(*args)
    return _prog_cache[key]


def _bf(x):
    return np.ascontiguousarray(np.asarray(x, dtype=np.float32)).astype(
        ml_dtypes.bfloat16)


def _rope_tables(S, dim):
    freqs = 1.0 / (THETA ** (np.arange(0, dim, 2, dtype=np.float32) / dim))
    f = np.arange(S, dtype=np.float32)[:, None] * freqs[None, :]
    cos = np.repeat(np.cos(f), 2, axis=-1).astype(np.float32)
    sin = np.repeat(np.sin(f), 2, axis=-1).astype(np.float32)
    return cos, sin


def _rot_lhsT(dim):
    rt = np.zeros((dim, dim), np.float32)
    for i in range(dim // 2):
        rt[2 * i + 1, 2 * i] = -1.0
        rt[2 * i, 2 * i + 1] = 1.0
    return rt


def _timed_run(nc, in_maps, reps=10):
    """Execute on all cores with device-resident inputs; time warm reps.

    Returns (results_list, best_exec_seconds). Mimics
    bass2jax.run_bass_via_pjrt's multi-core path but keeps inputs on
    device so the timed region is pure NEFF execution + dispatch.
    """
    import time as _time
    import jax
    from jax.sharding import Mesh, PartitionSpec, NamedSharding
    from jax.experimental.shard_map import shard_map
    from concourse import bass2jax as b2j
    from concourse import mybir as _mb

    b2j.install_neuronx_cc_hook()
    n_cores = len(in_maps)
    in_names, out_names, out_avals, zero_outs = [], [], [], []
    for alloc in nc.m.functions[0].allocations:
        if not isinstance(alloc, _mb.MemoryLocationSet):
            continue
        name = alloc.memorylocations[0].name
        pid_name = nc.partition_id_tensor.name if nc.partition_id_tensor else None
        if alloc.kind == "ExternalInput":
            if name != pid_name:
                in_names.append(name)
        elif alloc.kind == "ExternalOutput":
            out_names.append(name)
            shape = tuple(alloc.tensor_shape)
            dtype = _mb.dt.np(alloc.dtype)
            out_avals.append(jax.core.ShapedArray(shape, dtype))
            zero_outs.append(np.zeros(shape, dtype))
    n_params = len(in_names)
    n_outs = len(out_avals)
    all_names = list(in_names) + list(out_names)
    if nc.partition_id_tensor is not None:
        all_names.append(nc.partition_id_tensor.name)

    def _body(*args):
        operands = list(args)
        if nc.partition_id_tensor is not None:
            operands.append(b2j.partition_id_tensor())
        outs = b2j._bass_exec_p.bind(
            *operands, out_avals=tuple(out_avals), in_names=tuple(all_names),
            out_names=tuple(out_names), lowering_input_output_aliases=(),
            sim_require_finite=True, sim_require_nnan=True, nc=nc)
        return tuple(outs)

    devices = jax.devices()[:n_cores]
    mesh = Mesh(np.asarray(devices), ("core",))
    donate = tuple(range(n_params, n_params + n_outs))
    sharded = jax.jit(
        shard_map(_body, mesh=mesh,
                  in_specs=(PartitionSpec("core"),) * (n_params + n_outs),
                  out_specs=(PartitionSpec("core"),) * n_outs,
                  check_rep=False),
        donate_argnums=donate, keep_unused=True)
    sh = NamedSharding(mesh, PartitionSpec("core"))
    dev_in = [jax.device_put(
        np.concatenate([np.asarray(in_maps[c][n]) for c in range(n_cores)], axis=0), sh)
        for n in in_names]
    # warmup + correctness outputs
    dz = [jax.device_put(
        np.zeros((n_cores * z.shape[0], *z.shape[1:]), z.dtype), sh)
        for z in zero_outs]
    jax.block_until_ready(dz)
    outs = sharded(*dev_in, *dz)
    jax.block_until_ready(outs)
    # pipelined timing: amortize per-call dispatch overhead over reps
    zsets = [[jax.device_put(
        np.zeros((n_cores * z.shape[0], *z.shape[1:]), z.dtype), sh)
        for z in zero_outs] for _ in range(reps)]
    jax.block_until_ready(zsets)
    t0 = _time.perf_counter()
    last = None
    for k in range(reps):
        last = sharded(*dev_in, *zsets[k])
    jax.block_until_ready(last)
    total = _time.perf_counter() - t0
    best = total / reps
    results = [
        {name: np.asarray(outs[i]).reshape(n_cores, *out_avals[i].shape)[c]
         for i, name in enumerate(out_names)}
        for c in range(n_cores)]
    return results, best


_last_exec_ns = []


class _Res:
    def __init__(self, results):
        self.results = results


def _run(nc, in_maps, trace=False):
    try:
        from concourse._compat import axon_active
        use_timed = trace or axon_active()
    except ImportError:
        use_timed = trace
    if use_timed:
        # Under axon, run_bass_kernel_spmd(trace=False) cannot report
        # exec_time_ns; use the device-resident timed path instead.
        results, secs = _timed_run(nc, in_maps)
        _last_exec_ns.append(int(secs * 1e9))
        return _Res(results)
    res = run_bass_kernel_spmd(nc, in_maps, list(range(len(in_maps))))
    _last_exec_ns.append(res.exec_time_ns)
    return res


def kernel(query, key_value, g_q, g_kv, g_ffn, w_qc, w_kc, w_qr, w_kr, w_v,
           w_o, w_gate, w_up, w_down, _trace=False):
    query = np.asarray(query, np.float32)
    key_value = np.asarray(key_value, np.float32)
    Bq, Mq, _ = query.shape
    Nq = key_value.shape[1]
    NG = N_H // G2
    DC = D // P
    GH = G2 * D_H
    FC = FFN // P
    DTT = D // P
    TQ = NCORES // Bq                        # 4 token quarters per batch

    g_q = np.asarray(g_q, np.float32)[:, None]
    g_kv = np.asarray(g_kv, np.float32)[:, None]
    g_ffn = np.asarray(g_ffn, np.float32)[:, None]
    wqc = np.asarray(w_qc, np.float32) * g_q
    wqr_f = np.asarray(w_qr, np.float32) * g_q
    wkc = np.asarray(w_kc, np.float32) * g_kv
    wkr_f = np.asarray(w_kr, np.float32) * g_kv
    wv_f = np.asarray(w_v, np.float32) * g_kv
    wo_f = np.asarray(w_o, np.float32)
    wgate = np.asarray(w_gate, np.float32) * g_ffn
    wup = np.asarray(w_up, np.float32) * g_ffn
    wdown = np.asarray(w_down, np.float32)

    # packed weight layouts (contiguous per-group / per-chunk DMAs)
    wkp = _bf(wkc).reshape(DC, P, NG, GH).transpose(2, 1, 0, 3).reshape(NG, P, DC * GH).copy()
    wqp = _bf(wqc).reshape(DC, P, NG, GH).transpose(2, 1, 0, 3).reshape(NG, P, DC * GH).copy()
    wvp = _bf(wv_f).reshape(DC, P, NG, GH).transpose(2, 1, 0, 3).reshape(NG, P, DC * GH).copy()
    wkrp = _bf(wkr_f).reshape(DC, P, NG, P).transpose(2, 1, 0, 3).reshape(NG, P, DC * P).copy()
    wqrp = _bf(wqr_f).reshape(DC, P, NG, P).transpose(2, 1, 0, 3).reshape(NG, P, DC * P).copy()
    wop = _bf(wo_f).reshape(N_H, P, D).copy()
    wgp_ = _bf(wgate).reshape(DC, P, FC, P).transpose(2, 1, 0, 3).reshape(FC, P, DC * P).copy()
    wup_ = _bf(wup).reshape(DC, P, FC, P).transpose(2, 1, 0, 3).reshape(FC, P, DC * P).copy()
    wdp_ = _bf(wdown).reshape(FC, P, DTT, P).transpose(2, 1, 0, 3).reshape(DTT, P, FC * P).copy()

    cos, sin = _rope_tables(max(Mq, Nq), D_R)
    cosK = np.ascontiguousarray(np.vstack([cos[:Nq].T] * G2)).astype(np.float16)
    sinK = np.ascontiguousarray(np.vstack([sin[:Nq].T] * G2)).astype(np.float16)
    rot2 = _bf(np.kron(np.eye(G2, dtype=np.float32), _rot_lhsT(D_R)))

    kvT_b = [_bf(key_value[b].T) for b in range(Bq)]

    del _last_exec_ns[:]
    nc = _get(("fused", Nq, D), build_fused, Nq, D)
    in_maps = []
    for c in range(NCORES):
        b, tq = c // TQ, c % TQ
        sl = slice(tq * TOK, (tq + 1) * TOK)
        qs = query[b, sl, :]
        in_maps.append({
            "kvT": kvT_b[b],
            "qT": _bf(qs.T),
            "qres": np.ascontiguousarray(qs.T),
            "wkp": wkp, "wqp": wqp, "wvp": wvp,
            "wkrp": wkrp, "wqrp": wqrp, "wop": wop,
            "wgp": wgp_, "wup": wup_, "wdp": wdp_,
            "cosK": cosK, "sinK": sinK,
            "cosQ": np.ascontiguousarray(
                np.vstack([cos[sl].T] * G2)).astype(np.float16),
            "sinQ": np.ascontiguousarray(
                np.vstack([sin[sl].T] * G2)).astype(np.float16),
            "rot2": rot2,
        })
    res = _run(nc, in_maps, trace=_trace)

    y = np.empty((Bq, Mq, D), np.float32)
    for c in range(NCORES):
        b, tq = c // TQ, c % TQ
        y[b, tq * TOK:(tq + 1) * TOK, :] = res.results[c]["yT"].T
    return y
